# revision 70
# baseline (speedup 1.0000x reference)
"""Trainium2 Bass kernel for nn_DeformableRead (deformable attention read).

8 NeuronCores SPMD: core q -> batch q//4, anchor-cell rows 8*(q%4)..+8 (256
cells). Tokens routed to the core owning their anchor cell (host permutation).
Sample points live in fixed windows around each anchor cell (9x9/5x5/4x4 at
L2/L3/L4); bilinear sampling over a window is a dense 122-tap PE contraction
with separable hat weights relu(1-|x-i|) -- gather-free.

v3 (346us -> ~258us): patch blob padded to 128 partitions so each chunk DMA
spreads over all 16 SDMA engines (HWDGE splits a transfer across
gcd(outer_dim,16) engines; 122 rows -> only 2 engines at 26 GB/s = 253us DMA
critical path). Output DMA issued from sync engine (scalar is busy in phase
F). Pass C interleaved with phase F chunks (2-stage software skew) so the
hat/sampling pipeline starts after the first 512-slot block instead of after
all of pass C. Pass C writes tanh directly (clo folded into the iota table,
sigma applied on scalar as a per-partition-scale Copy activation). XU psum
evacuation merged 6->3 copies, od 2->1. Pass A square on gpsimd (idle in
lead-in). Last two chunks' hat/kappa ops biased to vector (drains ~10us
earlier than gpsimd, shortening the tail).
Measured engine quirks honored: f32 1x TT everywhere (bf16 strided
TT and 2-op tensor_scalar chains hit slow paths; gpsimd tensor_scalar is
~10x slower than DVE; matmul-transpose ignores its rhs values so no diag
scaling; DMA transpose from SBUF breaks).
Host does layout only: sharding, slot permutation, patch extraction, bf16
casts, fourier features of raw coords, constants. Device does all heavy math.
"""

import numpy as np
import ml_dtypes

import concourse.bass as bass
import concourse.bacc as bacc
import concourse.tile as tile
from concourse import mybir
from concourse.bass_utils import run_bass_kernel_spmd

D, H, NL, M = 192, 6, 3, 4
NF = 8
SIGMAS = (4.0, 2.0, 1.0)
WXY = (9, 5, 4)
CLO = (4.0, 2.0, 1.5)
PADL = (2, 1, 1)
SCALE = (4, 2, 1)
KWIN = sum(w * w for w in WXY)  # 122
LOFF = (0, WXY[0] ** 2, WXY[0] ** 2 + WXY[1] ** 2)
HATW = sum(4 * w for w in WXY)  # 72 per head per coord
HOFF = (0, 36, 56)
HATB = 6 * HATW  # 432 per coord
BF16 = mybir.dt.bfloat16
F32 = mybir.dt.float32

_CACHE = {}
VTAG = 15  # bump to invalidate terminal-side NEFF cache (shape-keyed)
KWPAD = 128  # patch partition dim padded 122->128: DMA splits across
             # gcd(outer_dim, 16) engines, so 122 -> only 2 engines


def _ap(base, free_off, dims):
    """Custom AP: base tile slice (sets partition range), explicit free dims."""
    return bass.AP(tensor=base.tensor, offset=base.offset + free_off,
                   ap=[base.ap[0]] + [list(d) for d in dims])


def _plan(top_indices):
    """Shared (cross-core) packing plan from top_indices."""
    ti = np.asarray(top_indices, np.int64)
    B, K, R = ti.shape
    counts = np.zeros((8, 256), np.int64)
    for q in range(8):
        b, crow = q // 4, q % 4
        cells = ti[b].reshape(-1)
        sel = cells[(cells >= crow * 256) & (cells < (crow + 1) * 256)] - crow * 256
        counts[q] = np.bincount(sel, minlength=256)
    order = np.argsort(counts, axis=1, kind='stable')  # per core: rank -> cell
    srt = np.sort(counts, axis=1)         # ascending: many-seg chunks first
    cap = srt.max(0)                      # capacity per rank
    cap = np.maximum(cap, 1)              # every rank owns >= 1 slot
    bnd = np.cumsum(cap)
    S2 = int(bnd[-1])
    NCH = (S2 + 127) // 128
    SP = NCH * 128
    # segments per chunk: (rank, s0_in_chunk, n)
    segs = [[] for _ in range(NCH)]
    for r in range(256):
        s0, s1 = int(bnd[r] - cap[r]), int(bnd[r])
        for ch in range(s0 // 128, (s1 - 1) // 128 + 1):
            a = max(s0, ch * 128)
            b_ = min(s1, (ch + 1) * 128)
            segs[ch].append((r, a - ch * 128, b_ - a))
    # extend final segment to cover padding tail
    if S2 < SP:
        r, a, n = segs[-1][-1]
        segs[-1][-1] = (r, a, n + SP - S2)
    return dict(counts=counts, order=order, cap=cap, bnd=bnd, S2=S2,
                SP=SP, NCH=NCH, segs=segs)


def _build_module(plan):
    SP, NCH, segs = plan['SP'], plan['NCH'], plan['segs']
    nsegtot = sum(len(s) for s in segs)
    nc = bacc.Bacc("TRN2", target_bir_lowering=False, debug=False)
    dt = nc.dram_tensor
    uinT = dt("uinT", [512, SP], BF16, kind="ExternalInput")
    pblob = dt("pblob", [KWPAD, nsegtot * D], BF16, kind="ExternalInput")
    wu = dt("wu", [416, D], BF16, kind="ExternalInput")
    wub = dt("wub", [D, 1], F32, kind="ExternalInput")
    wdaA = dt("wdaA", [96, 240], BF16, kind="ExternalInput")
    wdaB = dt("wdaB", [97, 240], BF16, kind="ExternalInput")
    bda = dt("bda", [112, 1], F32, kind="ExternalInput")
    bdb = dt("bdb", [32, 1], F32, kind="ExternalInput")
    blog = dt("blog", [72, 1], F32, kind="ExternalInput")
    bd6 = dt("bd6", [72, 72], BF16, kind="ExternalInput")
    sgA = dt("sgA", [112, 1], F32, kind="ExternalInput")
    sgB = dt("sgB", [32, 1], F32, kind="ExternalInput")
    iotah = dt("iotah", [128, 2 * HATB + VTAG], BF16, kind="ExternalInput")
    onesw = dt("onesw", [96, 96], BF16, kind="ExternalInput")
    identf = dt("identf", [128, 128], F32, kind="ExternalInput")
    identb = dt("identb", [128, 128], BF16, kind="ExternalInput")
    woA = dt("woA", [96, D], BF16, kind="ExternalInput")
    woB = dt("woB", [96, D], BF16, kind="ExternalInput")
    wob = dt("wob", [1, D], BF16, kind="ExternalInput")
    outT = dt("outT", [D, SP], F32, kind="ExternalOutput")

    NCS = [(i * 512, min(512, SP - i * 512)) for i in range((SP + 511) // 512)]
    AF = mybir.ActivationFunctionType
    OP = mybir.AluOpType

    with tile.TileContext(nc) as tc:
        with (
            tc.tile_pool(name="const", bufs=1) as cpool,
            tc.tile_pool(name="big", bufs=1) as bpool,
        ):
            _sbn = [0]
            def sb(t_ap, shape, dtype):
                _sbn[0] += 1
                nm = f"cst{_sbn[0]}"
                x = cpool.tile(shape, dtype, tag=nm, name=nm)
                nc.scalar.dma_start(x[:], t_ap)
                return x

            s_wu = []
            for kc in range(4):
                k0, k1 = kc * 128, min((kc + 1) * 128, 416)
                s_wu.append(sb(wu[k0:k1, :], [k1 - k0, D], BF16))
            s_wub = [sb(wub[0:96, :], [96, 1], F32), sb(wub[96:192, :], [96, 1], F32)]
            s_wdaA = sb(wdaA[:], [96, 240], BF16)
            s_wdaB = sb(wdaB[:], [97, 240], BF16)
            s_bda = sb(bda[:], [112, 1], F32)
            s_bdb = sb(bdb[:], [32, 1], F32)
            s_blog = sb(blog[:], [72, 1], F32)
            s_bd6 = sb(bd6[:], [72, 72], BF16)
            s_sgA = sb(sgA[:], [112, 1], F32)
            s_sgB = sb(sgB[:], [32, 1], F32)
            s_iota = sb(iotah[0:128, 0:2 * HATB], [128, 2 * HATB], BF16)
            s_ones = sb(onesw[:], [96, 96], BF16)
            s_idf = sb(identf[:], [128, 128], F32)
            s_idb = sb(identb[:], [128, 128], BF16)
            s_woA = sb(woA[:], [96, D], BF16)
            s_woB = sb(woB[:], [96, D], BF16)
            s_wob = sb(wob[:], [1, D], BF16)
            s_eps = cpool.tile([96, 1], F32, name="s_eps")
            nc.vector.memset(s_eps[:], 1e-5)
            s_one1 = cpool.tile([1, 128], BF16, name="s_one1")
            nc.vector.memset(s_one1[:], 1.0)

            # persistent activations
            yP = [bpool.tile([96, SP], BF16, tag="yP0", name="yP0"),
                  bpool.tile([96, SP], BF16, tag="yP1", name="yP1")]
            muP = bpool.tile([96, SP], F32, tag="muP")
            varP = bpool.tile([96, SP], F32, tag="varP")
            u0 = bpool.tile([96, SP], BF16, tag="u0", name="u0")
            u1 = bpool.tile([97, SP], BF16, tag="u1", name="u1")
            xaP = bpool.tile([112, SP], F32, tag="xaP")
            xbP = bpool.tile([32, SP], F32, tag="xbP")
            xwP = bpool.tile([72, SP], F32, tag="xwP")

            # ======== pass A: u matmul, gelu, stats  (gelu act table) ========
            with (
                tc.tile_pool(name="ucp", bufs=3) as ucpool,
                tc.tile_pool(name="wkA", bufs=2) as wpool,
                tc.tile_pool(name="psA", bufs=2, space="PSUM") as psA,
                tc.tile_pool(name="psB", bufs=2, space="PSUM") as psB,
            ):
                for n0, nn in NCS:
                    uc = ucpool.tile([128, 4, 512], BF16, tag="uc")
                    nc.sync.dma_start(
                        uc[:, :, :nn],
                        bass.AP(tensor=uinT[:].tensor, offset=n0,
                                ap=[[SP, 128], [128 * SP, 4], [1, nn]]))
                    pu = psA.tile([96, 2, 512], F32, tag="pu")
                    for mc in range(2):
                        for kc in range(4):
                            kk = min(128, 416 - kc * 128)
                            nc.tensor.matmul(
                                pu[:, mc, :nn],
                                s_wu[kc][:, mc * 96:(mc + 1) * 96],
                                uc[:kk, kc, :nn],
                                start=(kc == 0), stop=(kc == 3))
                        nc.scalar.activation(
                            out=yP[mc][:, n0:n0 + nn], in_=pu[:, mc, :nn],
                            func=AF.Gelu, bias=s_wub[mc], scale=1.0)
                    y2 = wpool.tile([96, 2, 512], BF16, tag="y2")
                    for mc in range(2):
                        nc.gpsimd.tensor_mul(
                            y2[:, mc, :nn], yP[mc][:, n0:n0 + nn],
                            yP[mc][:, n0:n0 + nn])
                    pst = psB.tile([96, 2, 512], F32, tag="pst")
                    nc.tensor.matmul(pst[:, 0, :nn], s_ones[:],
                                     yP[0][:, n0:n0 + nn], start=True, stop=False)
                    nc.tensor.matmul(pst[:, 0, :nn], s_ones[:],
                                     yP[1][:, n0:n0 + nn], start=False, stop=True)
                    nc.tensor.matmul(pst[:, 1, :nn], s_ones[:],
                                     y2[:, 0, :nn], start=True, stop=False)
                    nc.tensor.matmul(pst[:, 1, :nn], s_ones[:],
                                     y2[:, 1, :nn], start=False, stop=True)
                    nc.vector.tensor_scalar_mul(
                        out=muP[:, n0:n0 + nn], in0=pst[:, 0, :nn],
                        scalar1=1.0 / D)
                    musq = wpool.tile([96, 512], F32, tag="musq")
                    nc.gpsimd.tensor_mul(musq[:, :nn], muP[:, n0:n0 + nn],
                                         muP[:, n0:n0 + nn])
                    nc.vector.scalar_tensor_tensor(
                        out=varP[:, n0:n0 + nn], in0=pst[:, 1, :nn],
                        scalar=1.0 / D, in1=musq[:, :nn],
                        op0=OP.mult, op1=OP.subtract)

            # ======== pass B: rr = 1/sqrt(var+eps)  (sqrt act table) ========
            with tc.tile_pool(name="wkB", bufs=2) as wpool:
                for n0, nn in NCS:
                    sd = wpool.tile([96, 512], F32, tag="sd")
                    nc.scalar.activation(out=sd[:, :nn],
                                         in_=varP[:, n0:n0 + nn],
                                         func=AF.Sqrt, bias=s_eps, scale=1.0)
                    nc.vector.reciprocal_approx_fast(
                        out=varP[:, n0:n0 + nn], in_=sd[:, :nn])

            # ======== pass C (per 512 block) interleaved with phase F ========
            with (
                tc.tile_pool(name="wkC", bufs=2) as wpool,
                tc.tile_pool(name="psC", bufs=1, space="PSUM") as psC,
                tc.tile_pool(name="psD2", bufs=1, space="PSUM") as psD2,
                tc.tile_pool(name="psE", bufs=1, space="PSUM") as psE,
                tc.tile_pool(name="kw", bufs=4) as kpool,
                tc.tile_pool(name="pp", bufs=6) as ppool,
                tc.tile_pool(name="psT", bufs=1, space="PSUM") as psT,
                tc.tile_pool(name="psK", bufs=1, space="PSUM") as psK,
                tc.tile_pool(name="psX", bufs=1, space="PSUM") as psX,
                tc.tile_pool(name="psDo", bufs=1, space="PSUM") as psDo,
            ):
                def passC(n0, nn):
                    nc.vector.tensor_mul(u0[:, n0:n0 + nn],
                                         yP[0][:, n0:n0 + nn],
                                         varP[:, n0:n0 + nn])
                    nc.gpsimd.tensor_mul(u1[0:96, n0:n0 + nn],
                                         yP[1][:, n0:n0 + nn],
                                         varP[:, n0:n0 + nn])
                    nc.vector.tensor_mul(u1[96:97, n0:n0 + nn],
                                         muP[0:1, n0:n0 + nn],
                                         varP[0:1, n0:n0 + nn])
                    pdc = psC.tile([112, 512], F32, tag="pdc")
                    nc.tensor.matmul(pdc[:, :nn], s_wdaA[:, 0:112],
                                     u0[:, n0:n0 + nn], start=True, stop=False)
                    nc.tensor.matmul(pdc[:, :nn], s_wdaB[:, 0:112],
                                     u1[:, n0:n0 + nn], start=False, stop=True)
                    pdd = psD2.tile([128, 512], F32, tag="pdd")
                    nc.tensor.matmul(pdd[:, :nn], s_wdaA[:, 112:240],
                                     u0[:, n0:n0 + nn], start=True, stop=False)
                    nc.tensor.matmul(pdd[:, :nn], s_wdaB[:, 112:240],
                                     u1[:, n0:n0 + nn], start=False, stop=True)
                    nc.scalar.activation(out=xaP[:, n0:n0 + nn],
                                         in_=pdc[:, :nn],
                                         func=AF.Tanh, bias=s_bda, scale=1.0)
                    nc.scalar.activation(out=xbP[:, n0:n0 + nn],
                                         in_=pdd[96:128, :nn],
                                         func=AF.Tanh, bias=s_bdb, scale=1.0)
                    nc.scalar.activation(out=xaP[:, n0:n0 + nn],
                                         in_=xaP[:, n0:n0 + nn],
                                         func=AF.Copy, scale=s_sgA)
                    nc.scalar.activation(out=xbP[:, n0:n0 + nn],
                                         in_=xbP[:, n0:n0 + nn],
                                         func=AF.Copy, scale=s_sgB)
                    exw = wpool.tile([72, 512], BF16, tag="exw")
                    nc.scalar.activation(out=exw[:, :nn], in_=pdd[0:72, :nn],
                                         func=AF.Exp, bias=s_blog, scale=1.0)
                    pz = psE.tile([72, 512], F32, tag="pz")
                    nc.tensor.matmul(pz[:, :nn], s_bd6[:], exw[:, :nn],
                                     start=True, stop=True)
                    rz = wpool.tile([72, 512], F32, tag="rz")
                    nc.vector.reciprocal_approx_fast(out=rz[:, :nn],
                                                     in_=pz[:, :nn])
                    nc.vector.tensor_mul(xwP[:, n0:n0 + nn], exw[:, :nn],
                                         rz[:, :nn])

                # ======== phase F: hats, kappa, sampling, w_o ========
                pcolv = [0]
                def S1(q):
                    c0 = q * 128
                    sg = segs[q]
                    nseg = len(sg)
                    st = {}
                    pT = psT.tile([128, 216], F32, tag="pT", name="pT")
                    nc.tensor.transpose(pT[:, 0:112], xaP[:, c0:c0 + 128],
                                        s_idf[:112, :112])
                    nc.tensor.transpose(pT[:, 112:144], xbP[:, c0:c0 + 128],
                                        s_idf[:32, :32])
                    nc.tensor.transpose(pT[:, 144:216], xwP[:, c0:c0 + 128],
                                        s_idf[:72, :72])
                    rm = kpool.tile([128, 216], F32, tag="rm", name="rm")
                    nc.scalar.copy(out=rm[:, 0:144], in_=pT[:, 0:144])
                    nc.scalar.copy(out=rm[:, 144:216], in_=pT[:, 144:216])
                    patch = ppool.tile([KWPAD, nseg * D], BF16, tag="patch",
                                       name="patch")
                    pcol = pcolv[0]
                    nc.sync.dma_start(patch[:],
                                      pblob[:, pcol * D:(pcol + nseg) * D])
                    pcolv[0] += nseg
                    hxy = kpool.tile([128, 2 * HATB], F32, tag="hxy",
                                     name="hxy")
                    for coord in range(2):
                        eng = nc.vector if coord == 0 else nc.gpsimd
                        for l in range(NL):
                            w = WXY[l]
                            out_ap = _ap(hxy[:], coord * HATB + HOFF[l],
                                         [[72, 6], [w, 4], [1, w]])
                            in0 = _ap(rm[:], 8 * l + coord,
                                      [[24, 6], [2, 4], [0, w]])
                            in1 = _ap(s_iota[:], coord * HATB + HOFF[l],
                                      [[72, 6], [w, 4], [1, w]])
                            eng.tensor_sub(out_ap, in0, in1)
                    st['rm'], st['hxy'], st['patch'] = rm, hxy, patch
                    return st

                def S2a(q, st):
                    hs = st['hxy'][:]
                    nc.scalar.activation(out=hs, in_=hs, func=AF.Abs)
                    nc.scalar.activation(out=hs, in_=hs, func=AF.Relu,
                                         bias=1.0, scale=-1.0)

                def S2(q, st):
                    rm, hxy = st['rm'], st['hxy']
                    tail = q >= NCH - 2   # vector drains last chunks
                    for l in range(NL):
                        w = WXY[l]
                        hy_ap = _ap(hxy[:], HATB + HOFF[l],
                                    [[72, 6], [w, 4], [1, w]])
                        wt_ap = _ap(rm[:], 144 + 4 * l,
                                    [[12, 6], [1, 4], [0, w]])
                        eng = nc.gpsimd if (l == 0 and not tail) else nc.vector
                        eng.tensor_mul(hy_ap, hy_ap, wt_ap)
                    kap = kpool.tile([128, 6 * KWIN], BF16, tag="kap",
                                     name="kap")
                    tmp = kpool.tile([128, 6 * 4 * WXY[0] ** 2], F32,
                                     tag="tmp", name="tmp")
                    for l in range(NL):
                        w = WXY[l]
                        for m in range(4):
                            hy = _ap(hxy[:], HATB + HOFF[l] + m * w,
                                     [[72, 6], [1, w], [0, w]])
                            hx = _ap(hxy[:], HOFF[l] + m * w,
                                     [[72, 6], [0, w], [1, w]])
                            t1 = _ap(tmp[:], m * w * w,
                                     [[4 * w * w, 6], [w, w], [1, w]])
                            eng = (nc.gpsimd if m == 3 else nc.vector
                                   ) if tail else (
                                   nc.gpsimd if m % 2 else nc.vector)
                            eng.tensor_mul(t1, hy, hx)
                        t2a = _ap(tmp[:], 0,
                                  [[4 * w * w, 6], [w * w, 2], [w, w], [1, w]])
                        t2b = _ap(tmp[:], 2 * w * w,
                                  [[4 * w * w, 6], [w * w, 2], [w, w], [1, w]])
                        eng = nc.vector if (l == 0 or tail) else nc.gpsimd
                        eng.tensor_add(t2a, t2a, t2b)
                        ksl = _ap(kap[:], LOFF[l], [[KWIN, 6], [w, w], [1, w]])
                        t1a = _ap(tmp[:], 0, [[4 * w * w, 6], [w, w], [1, w]])
                        t1b = _ap(tmp[:], w * w,
                                  [[4 * w * w, 6], [w, w], [1, w]])
                        eng = nc.vector if tail else (
                            nc.gpsimd if l == 0 else nc.vector)
                        eng.tensor_add(ksl, t1a, t1b)
                    st['kap'] = kap

                def S3(q, st):
                    c0 = q * 128
                    sg = segs[q]
                    kap, patch = st['kap'], st['patch']
                    pK = psK.tile([122, 6, 128], BF16, tag="pK", name="pK")
                    for hh in range(H):
                        nc.tensor.transpose(pK[:, hh, :],
                                            kap[:, hh * KWIN:(hh + 1) * KWIN],
                                            s_idb[:])
                    kT = kpool.tile([122, 6, 128], BF16, tag="kT", name="kT")
                    nc.scalar.copy(out=kT[:, 0:3, :], in_=pK[:, 0:3, :])
                    nc.vector.tensor_copy(kT[:, 3:6, :], pK[:, 3:6, :])
                    pXt = psX.tile([96, 8, 128], F32, tag="pXt", name="pXt")
                    pXa = pXt[:, 0:3, :]
                    pXb = pXt[:, 4:7, :]
                    for j, (r, s0, n) in enumerate(sg):
                        nc.tensor.matmul(
                            pXa[:, :, s0:s0 + n],
                            patch[0:KWIN, j * D:j * D + 96],
                            kT[:, 0:3, s0:s0 + n],
                            start=True, stop=True)
                        nc.tensor.matmul(
                            pXb[:, :, s0:s0 + n],
                            patch[0:KWIN, j * D + 96:j * D + 192],
                            kT[:, 3:6, s0:s0 + n],
                            start=True, stop=True)
                    XU = kpool.tile([96, 2, 128], BF16, tag="XU", name="XU")
                    for hh in range(3):
                        base = pXt[32 * hh:32 * hh + 32, 0, :]
                        nc.scalar.copy(
                            out=XU[32 * hh:32 * hh + 32, :, :],
                            in_=_ap(base, hh * 128, [[512, 2], [1, 128]]))
                    pDt = psDo.tile([96, 2, 128], F32, tag="pDt", name="pDt")
                    od = kpool.tile([96, 2, 128], F32, tag="od", name="od")
                    for mc in range(2):
                        nc.tensor.matmul(pDt[:, mc, :],
                                         s_woA[:, mc * 96:(mc + 1) * 96],
                                         XU[:, 0, :], start=True, stop=False)
                        nc.tensor.matmul(pDt[:, mc, :],
                                         s_woB[:, mc * 96:(mc + 1) * 96],
                                         XU[:, 1, :], start=False, stop=False)
                        nc.tensor.matmul(pDt[:, mc, :],
                                         s_wob[:, mc * 96:(mc + 1) * 96],
                                         s_one1[:], start=False, stop=True)
                    nc.scalar.copy(out=od[:], in_=pDt[:])
                    nc.sync.dma_start(
                        bass.AP(tensor=outT[:].tensor, offset=c0,
                                ap=[[SP, 96], [96 * SP, 2], [1, 128]]),
                        od[:])

                # interleave pass C blocks with a 2-stage software skew of
                # phase F (engine queues are in-order; interleaving chunks
                # fills cross-engine handoff bubbles, and starting F right
                # after C(0) overlaps the lead-in)
                nblk = len(NCS)
                blk_end = [(n0 + nn) // 128 for n0, nn in NCS]
                emitted_c = [0]
                def needC(t):
                    while emitted_c[0] < nblk and (
                            0 if emitted_c[0] == 0
                            else blk_end[emitted_c[0] - 1]) < t + 1:
                        j = emitted_c[0]
                        passC(NCS[j][0], NCS[j][1])
                        emitted_c[0] += 1
                sts = [None] * NCH
                for t in range(NCH + 3):
                    if t < NCH:
                        needC(min(t + 1, NCH - 1))
                        sts[t] = S1(t)
                    if 0 <= t - 2 < NCH:
                        S2a(t - 2, sts[t - 2])
                        S2(t - 2, sts[t - 2])
                    if t - 3 >= 0:
                        S3(t - 3, sts[t - 3])
    nc.compile()
    return nc


def _host_prep(inputs, plan):
    h = inputs["h"].astype(np.float32)
    ti = np.asarray(inputs["top_indices"], np.int64)
    qc = inputs["query_coords"].astype(np.float32)
    g = inputs["g"].astype(np.float32)
    maps = [np.asarray(inputs["L2_proj"], np.float32),
            np.asarray(inputs["L3_proj"], np.float32),
            np.asarray(inputs["L4_proj"], np.float32)]
    B, K, R = ti.shape
    cap, bnd, SP, NCH, segs = (plan['cap'], plan['bnd'], plan['SP'],
                               plan['NCH'], plan['segs'])
    order = plan['order']

    consts = {}
    consts["wu"] = np.ascontiguousarray(inputs["w_u_w"].T).astype(ml_dtypes.bfloat16)
    consts["wub"] = inputs["w_u_b"].reshape(D, 1).astype(np.float32)
    # LN fold: z = Wg.(y*rr) - rowsum(Wg).(mu*rr) + (W.b + c)
    gam = inputs["ln_u_g"].astype(np.float32)
    bet = inputs["ln_u_b"].astype(np.float32)
    Wall = np.concatenate([inputs["w_delta_w"], inputs["w_a_w"]], 0)  # [216,192]
    ball = np.concatenate([inputs["w_delta_b"], inputs["w_a_b"]], 0)  # [216]
    Wg = Wall * gam[None, :]
    Wg240 = np.zeros((240, D), np.float32)
    Wg240[0:112] = Wg[0:112]
    Wg240[112:184] = Wg[144:216]
    Wg240[208:240] = Wg[112:144]
    lhs = np.concatenate([Wg240.T, -Wg240.sum(1)[None, :]], 0)  # [193, 240]
    consts["wdaA"] = lhs[0:96].astype(ml_dtypes.bfloat16)
    consts["wdaB"] = lhs[96:193].astype(ml_dtypes.bfloat16)
    biasf = Wall @ bet + ball                              # [216]
    consts["bda"] = biasf[0:112].reshape(112, 1).astype(np.float32)
    consts["bdb"] = biasf[112:144].reshape(32, 1).astype(np.float32)
    consts["blog"] = biasf[144:216].reshape(72, 1).astype(np.float32)
    consts["bd6"] = np.kron(np.eye(H, dtype=np.float32),
                            np.ones((12, 12), np.float32)).astype(ml_dtypes.bfloat16)
    # per-offset-row sigma (rows (h,l,m,c): l = (o//8)%3)
    sv = np.array([SIGMAS[(o // 8) % 3] for o in range(144)], np.float32)
    consts["sgA"] = sv[0:112].reshape(112, 1)
    consts["sgB"] = sv[112:144].reshape(32, 1)
    # iota: (i - clo); device x = sig*tanh, so hat = relu(1-|x - iota|)
    io = np.zeros((128, 2 * HATB + VTAG), np.float32)
    for coord in range(2):
        for l in range(NL):
            w = WXY[l]
            for hh in range(H):
                for m in range(M):
                    st = coord * HATB + HOFF[l] + 72 * hh + w * m
                    io[:, st:st + w] = np.arange(w, dtype=np.float32) - CLO[l]
    consts["iotah"] = io.astype(ml_dtypes.bfloat16)
    consts["onesw"] = np.ones((96, 96), ml_dtypes.bfloat16)
    consts["identf"] = np.eye(128, dtype=np.float32)
    consts["identb"] = np.eye(128, dtype=ml_dtypes.bfloat16)
    woT = np.ascontiguousarray(inputs["w_o_w"].T).astype(np.float32)
    consts["woA"] = woT[0:96].astype(ml_dtypes.bfloat16)
    consts["woB"] = woT[96:192].astype(ml_dtypes.bfloat16)
    consts["wob"] = (inputs["w_o_b"] + inputs["e_deform"].reshape(-1)
                     ).reshape(1, D).astype(ml_dtypes.bfloat16)

    pmaps = []
    for b in range(B):
        pm = []
        for l in range(NL):
            Wl = maps[l].shape[3]
            mp = np.transpose(maps[l][b], (1, 2, 0))
            Hp = 32 * SCALE[l] + WXY[l]
            out = np.zeros((Hp, Hp, D), np.float32)
            out[PADL[l]:PADL[l] + Wl, PADL[l]:PADL[l] + Wl] = mp
            pm.append(out.astype(ml_dtypes.bfloat16))
        pmaps.append(pm)

    freqs = 2.0 ** np.arange(NF, dtype=np.float32)
    cell_of = ti.reshape(B, K * R)

    in_maps, slot_maps = [], []
    for q in range(8):
        b, crow = q // 4, q % 4
        d = dict(consts)
        # rank -> cell (local id in 0..255), patches in per-chunk segment order
        r2c = order[q]
        cells_seq = []
        for ch in range(NCH):
            for (r, s0, n) in segs[ch]:
                cells_seq.append(r2c[r])
        pats_all = np.zeros((KWPAD, len(cells_seq) * D), ml_dtypes.bfloat16)
        for j, cid in enumerate(cells_seq):
            ayc, axc = cid // 32, cid % 32
            col = []
            for l in range(NL):
                w = WXY[l]
                pm = pmaps[b][l]
                r0 = SCALE[l] * 8 * crow
                ys = (r0 + SCALE[l] * ayc) + np.arange(w)
                xs = (SCALE[l] * axc) + np.arange(w)
                pt = pm[ys[:, None], xs[None, :], :]     # [w, w, D]
                col.append(pt.reshape(w * w, D))
            pats_all[:KWIN, j * D:(j + 1) * D] = np.concatenate(col, 0)
        d["pblob"] = pats_all

        # slot -> token
        slot_tok = -np.ones(SP, np.int64)
        cnt = plan['counts'][q]
        for r in range(256):
            cid = r2c[r]
            gcid = crow * 256 + cid
            toks = np.nonzero(cell_of[b] == gcid)[0]
            s0 = int(bnd[r] - cap[r])
            assert len(toks) <= cap[r]
            slot_tok[s0:s0 + len(toks)] = toks
        valid = slot_tok >= 0
        st = np.where(valid, slot_tok, 0)
        k_of = st // R
        cid_of = cell_of[b][st]
        h_s = h[b][k_of] * valid[:, None]
        g_s = g[b][cid_of] * valid[:, None]
        qc_s = qc[b][k_of]
        ax = (cid_of % 32).astype(np.float32)
        ay = (cid_of // 32).astype(np.float32)
        anchor = np.stack([ax * 32 + 16, ay * 32 + 16], -1)
        dp = (anchor - qc_s) / 1024.0
        xf = dp[:, 0:1] * freqs * 2 * np.pi
        yf = dp[:, 1:2] * freqs * 2 * np.pi
        phi = np.concatenate([np.sin(xf), np.cos(xf), np.sin(yf), np.cos(yf)],
                             -1).astype(np.float32) * valid[:, None]
        u_in = np.concatenate([h_s, g_s, phi], -1)
        uT = np.zeros((512, SP), ml_dtypes.bfloat16)
        uT[0:416] = np.ascontiguousarray(u_in.T).astype(ml_dtypes.bfloat16)
        d["uinT"] = uT
        in_maps.append(d)
        slot_maps.append((slot_tok, valid))
    return in_maps, slot_maps


def kernel(**inputs):
    plan = _plan(inputs["top_indices"])
    key = plan['SP'], tuple(plan['cap'].tolist())
    if _CACHE.get("key") != key:
        _CACHE["nc"] = _build_module(plan)
        _CACHE["key"] = key
    nc = _CACHE["nc"]
    in_maps, slot_maps = _host_prep(inputs, plan)
    res = run_bass_kernel_spmd(nc, in_maps, core_ids=list(range(8)),
                               **_CACHE.get("run_kwargs", {}))
    _CACHE["last"] = res
    B, K, R = inputs["top_indices"].shape
    out = np.zeros((B, K * R, D), np.float32)
    for q in range(8):
        b = q // 4
        oT = np.asarray(res.results[q]["outT"], np.float32)
        slot_tok, valid = slot_maps[q]
        out[b, slot_tok[valid]] = oT.T[valid]
    return out.reshape(B, K, R, D)


# revision 71
# speedup vs baseline: 1.0036x; 1.0036x over previous
"""Trainium2 Bass kernel for nn_DeformableRead (deformable attention read).

8 NeuronCores SPMD: core q -> batch q//4, anchor-cell rows 8*(q%4)..+8 (256
cells). Tokens routed to the core owning their anchor cell (host permutation).
Sample points live in fixed windows around each anchor cell (9x9/5x5/4x4 at
L2/L3/L4); bilinear sampling over a window is a dense 122-tap PE contraction
with separable hat weights relu(1-|x-i|) -- gather-free.

v3 (346us -> ~258us): patch blob padded to 128 partitions so each chunk DMA
spreads over all 16 SDMA engines (HWDGE splits a transfer across
gcd(outer_dim,16) engines; 122 rows -> only 2 engines at 26 GB/s = 253us DMA
critical path). Output DMA issued from sync engine (scalar is busy in phase
F). Pass C interleaved with phase F chunks (2-stage software skew) so the
hat/sampling pipeline starts after the first 512-slot block instead of after
all of pass C. Pass C writes tanh directly (clo folded into the iota table,
sigma applied on scalar as a per-partition-scale Copy activation). XU psum
evacuation merged 6->3 copies, od 2->1. Pass A square on gpsimd (idle in
lead-in). Last two chunks' hat/kappa ops biased to vector (drains ~10us
earlier than gpsimd, shortening the tail).
Measured engine quirks honored: f32 1x TT everywhere (bf16 strided
TT and 2-op tensor_scalar chains hit slow paths; gpsimd tensor_scalar is
~10x slower than DVE; matmul-transpose ignores its rhs values so no diag
scaling; DMA transpose from SBUF breaks).
Host does layout only: sharding, slot permutation, patch extraction, bf16
casts, fourier features of raw coords, constants. Device does all heavy math.
"""

import numpy as np
import ml_dtypes

import concourse.bass as bass
import concourse.bacc as bacc
import concourse.tile as tile
from concourse import mybir
from concourse.bass_utils import run_bass_kernel_spmd

D, H, NL, M = 192, 6, 3, 4
NF = 8
SIGMAS = (4.0, 2.0, 1.0)
WXY = (9, 5, 4)
CLO = (4.0, 2.0, 1.5)
PADL = (2, 1, 1)
SCALE = (4, 2, 1)
KWIN = sum(w * w for w in WXY)  # 122
LOFF = (0, WXY[0] ** 2, WXY[0] ** 2 + WXY[1] ** 2)
HATW = sum(4 * w for w in WXY)  # 72 per head per coord
HOFF = (0, 36, 56)
HATB = 6 * HATW  # 432 per coord
BF16 = mybir.dt.bfloat16
F32 = mybir.dt.float32

_CACHE = {}
VTAG = 15  # bump to invalidate terminal-side NEFF cache (shape-keyed)
KWPAD = 128  # patch partition dim padded 122->128: DMA splits across
             # gcd(outer_dim, 16) engines, so 122 -> only 2 engines


def _ap(base, free_off, dims):
    """Custom AP: base tile slice (sets partition range), explicit free dims."""
    return bass.AP(tensor=base.tensor, offset=base.offset + free_off,
                   ap=[base.ap[0]] + [list(d) for d in dims])


def _plan(top_indices):
    """Shared (cross-core) packing plan from top_indices."""
    ti = np.asarray(top_indices, np.int64)
    B, K, R = ti.shape
    counts = np.zeros((8, 256), np.int64)
    for q in range(8):
        b, crow = q // 4, q % 4
        cells = ti[b].reshape(-1)
        sel = cells[(cells >= crow * 256) & (cells < (crow + 1) * 256)] - crow * 256
        counts[q] = np.bincount(sel, minlength=256)
    order = np.argsort(counts, axis=1, kind='stable')  # per core: rank -> cell
    srt = np.sort(counts, axis=1)         # ascending: many-seg chunks first
    cap = srt.max(0)                      # capacity per rank
    cap = np.maximum(cap, 1)              # every rank owns >= 1 slot
    bnd = np.cumsum(cap)
    S2 = int(bnd[-1])
    NCH = (S2 + 127) // 128
    SP = NCH * 128
    # segments per chunk: (rank, s0_in_chunk, n)
    segs = [[] for _ in range(NCH)]
    for r in range(256):
        s0, s1 = int(bnd[r] - cap[r]), int(bnd[r])
        for ch in range(s0 // 128, (s1 - 1) // 128 + 1):
            a = max(s0, ch * 128)
            b_ = min(s1, (ch + 1) * 128)
            segs[ch].append((r, a - ch * 128, b_ - a))
    # extend final segment to cover padding tail
    if S2 < SP:
        r, a, n = segs[-1][-1]
        segs[-1][-1] = (r, a, n + SP - S2)
    return dict(counts=counts, order=order, cap=cap, bnd=bnd, S2=S2,
                SP=SP, NCH=NCH, segs=segs)


def _build_module(plan):
    SP, NCH, segs = plan['SP'], plan['NCH'], plan['segs']
    nsegtot = sum(len(s) for s in segs)
    nc = bacc.Bacc("TRN2", target_bir_lowering=False, debug=False)
    dt = nc.dram_tensor
    uinT = dt("uinT", [512, SP], BF16, kind="ExternalInput")
    pblob = dt("pblob", [KWPAD, nsegtot * D], BF16, kind="ExternalInput")
    wu = dt("wu", [416, D], BF16, kind="ExternalInput")
    wub = dt("wub", [D, 1], F32, kind="ExternalInput")
    wdaA = dt("wdaA", [96, 240], BF16, kind="ExternalInput")
    wdaB = dt("wdaB", [97, 240], BF16, kind="ExternalInput")
    bda = dt("bda", [112, 1], F32, kind="ExternalInput")
    bdb = dt("bdb", [32, 1], F32, kind="ExternalInput")
    blog = dt("blog", [72, 1], F32, kind="ExternalInput")
    bd6 = dt("bd6", [72, 72], BF16, kind="ExternalInput")
    sgA = dt("sgA", [112, 1], F32, kind="ExternalInput")
    sgB = dt("sgB", [32, 1], F32, kind="ExternalInput")
    iotah = dt("iotah", [128, 2 * HATB + VTAG], BF16, kind="ExternalInput")
    onesw = dt("onesw", [96, 96], BF16, kind="ExternalInput")
    identf = dt("identf", [128, 128], F32, kind="ExternalInput")
    identb = dt("identb", [128, 128], BF16, kind="ExternalInput")
    woA = dt("woA", [96, D], BF16, kind="ExternalInput")
    woB = dt("woB", [96, D], BF16, kind="ExternalInput")
    wob = dt("wob", [1, D], BF16, kind="ExternalInput")
    outT = dt("outT", [D, SP], F32, kind="ExternalOutput")

    NCS = [(i * 512, min(512, SP - i * 512)) for i in range((SP + 511) // 512)]
    AF = mybir.ActivationFunctionType
    OP = mybir.AluOpType

    with tile.TileContext(nc) as tc:
        with (
            tc.tile_pool(name="const", bufs=1) as cpool,
            tc.tile_pool(name="big", bufs=1) as bpool,
        ):
            _sbn = [0]
            def sb(t_ap, shape, dtype):
                _sbn[0] += 1
                nm = f"cst{_sbn[0]}"
                x = cpool.tile(shape, dtype, tag=nm, name=nm)
                nc.scalar.dma_start(x[:], t_ap)
                return x

            s_wu = []
            for kc in range(4):
                k0, k1 = kc * 128, min((kc + 1) * 128, 416)
                s_wu.append(sb(wu[k0:k1, :], [k1 - k0, D], BF16))
            s_wub = [sb(wub[0:96, :], [96, 1], F32), sb(wub[96:192, :], [96, 1], F32)]
            s_wdaA = sb(wdaA[:], [96, 240], BF16)
            s_wdaB = sb(wdaB[:], [97, 240], BF16)
            s_bda = sb(bda[:], [112, 1], F32)
            s_bdb = sb(bdb[:], [32, 1], F32)
            s_blog = sb(blog[:], [72, 1], F32)
            s_bd6 = sb(bd6[:], [72, 72], BF16)
            s_sgA = sb(sgA[:], [112, 1], F32)
            s_sgB = sb(sgB[:], [32, 1], F32)
            s_iota = sb(iotah[0:128, 0:2 * HATB], [128, 2 * HATB], BF16)
            s_ones = sb(onesw[:], [96, 96], BF16)
            s_idf = sb(identf[:], [128, 128], F32)
            s_idb = sb(identb[:], [128, 128], BF16)
            s_woA = sb(woA[:], [96, D], BF16)
            s_woB = sb(woB[:], [96, D], BF16)
            s_wob = sb(wob[:], [1, D], BF16)
            s_eps = cpool.tile([96, 1], F32, name="s_eps")
            nc.vector.memset(s_eps[:], 1e-5)
            s_one1 = cpool.tile([1, 128], BF16, name="s_one1")
            nc.vector.memset(s_one1[:], 1.0)

            # persistent activations
            yP = [bpool.tile([96, SP], BF16, tag="yP0", name="yP0"),
                  bpool.tile([96, SP], BF16, tag="yP1", name="yP1")]
            muP = bpool.tile([96, SP], F32, tag="muP")
            varP = bpool.tile([96, SP], F32, tag="varP")
            u0 = bpool.tile([96, SP], BF16, tag="u0", name="u0")
            u1 = bpool.tile([97, SP], BF16, tag="u1", name="u1")
            xaP = bpool.tile([112, SP], F32, tag="xaP")
            xbP = bpool.tile([32, SP], F32, tag="xbP")
            xwP = bpool.tile([72, SP], F32, tag="xwP")

            # ======== pass A: u matmul, gelu, stats  (gelu act table) ========
            with (
                tc.tile_pool(name="ucp", bufs=3) as ucpool,
                tc.tile_pool(name="wkA", bufs=2) as wpool,
                tc.tile_pool(name="psA", bufs=2, space="PSUM") as psA,
                tc.tile_pool(name="psB", bufs=2, space="PSUM") as psB,
            ):
                for n0, nn in NCS:
                    uc = ucpool.tile([128, 4, 512], BF16, tag="uc")
                    nc.sync.dma_start(
                        uc[:, :, :nn],
                        bass.AP(tensor=uinT[:].tensor, offset=n0,
                                ap=[[SP, 128], [128 * SP, 4], [1, nn]]))
                    pu = psA.tile([96, 2, 512], F32, tag="pu")
                    for mc in range(2):
                        for kc in range(4):
                            kk = min(128, 416 - kc * 128)
                            nc.tensor.matmul(
                                pu[:, mc, :nn],
                                s_wu[kc][:, mc * 96:(mc + 1) * 96],
                                uc[:kk, kc, :nn],
                                start=(kc == 0), stop=(kc == 3))
                        nc.scalar.activation(
                            out=yP[mc][:, n0:n0 + nn], in_=pu[:, mc, :nn],
                            func=AF.Gelu, bias=s_wub[mc], scale=1.0)
                    y2 = wpool.tile([96, 2, 512], BF16, tag="y2")
                    for mc in range(2):
                        nc.gpsimd.tensor_mul(
                            y2[:, mc, :nn], yP[mc][:, n0:n0 + nn],
                            yP[mc][:, n0:n0 + nn])
                    pst = psB.tile([96, 2, 512], F32, tag="pst")
                    nc.tensor.matmul(pst[:, 0, :nn], s_ones[:],
                                     yP[0][:, n0:n0 + nn], start=True, stop=False)
                    nc.tensor.matmul(pst[:, 0, :nn], s_ones[:],
                                     yP[1][:, n0:n0 + nn], start=False, stop=True)
                    nc.tensor.matmul(pst[:, 1, :nn], s_ones[:],
                                     y2[:, 0, :nn], start=True, stop=False)
                    nc.tensor.matmul(pst[:, 1, :nn], s_ones[:],
                                     y2[:, 1, :nn], start=False, stop=True)
                    nc.vector.tensor_scalar_mul(
                        out=muP[:, n0:n0 + nn], in0=pst[:, 0, :nn],
                        scalar1=1.0 / D)
                    musq = wpool.tile([96, 512], F32, tag="musq")
                    nc.gpsimd.tensor_mul(musq[:, :nn], muP[:, n0:n0 + nn],
                                         muP[:, n0:n0 + nn])
                    nc.vector.scalar_tensor_tensor(
                        out=varP[:, n0:n0 + nn], in0=pst[:, 1, :nn],
                        scalar=1.0 / D, in1=musq[:, :nn],
                        op0=OP.mult, op1=OP.subtract)

            # ======== pass B: rr = 1/sqrt(var+eps)  (sqrt act table) ========
            with tc.tile_pool(name="wkB", bufs=2) as wpool:
                for n0, nn in NCS:
                    sd = wpool.tile([96, 512], F32, tag="sd")
                    nc.scalar.activation(out=sd[:, :nn],
                                         in_=varP[:, n0:n0 + nn],
                                         func=AF.Sqrt, bias=s_eps, scale=1.0)
                    nc.vector.reciprocal_approx_fast(
                        out=varP[:, n0:n0 + nn], in_=sd[:, :nn])

            # ======== pass C (per 512 block) interleaved with phase F ========
            with (
                tc.tile_pool(name="wkC", bufs=2) as wpool,
                tc.tile_pool(name="psC", bufs=1, space="PSUM") as psC,
                tc.tile_pool(name="psD2", bufs=1, space="PSUM") as psD2,
                tc.tile_pool(name="psE", bufs=1, space="PSUM") as psE,
                tc.tile_pool(name="kw", bufs=4) as kpool,
                tc.tile_pool(name="pp", bufs=6) as ppool,
                tc.tile_pool(name="psT", bufs=1, space="PSUM") as psT,
                tc.tile_pool(name="psK", bufs=1, space="PSUM") as psK,
                tc.tile_pool(name="psX", bufs=1, space="PSUM") as psX,
                tc.tile_pool(name="psDo", bufs=1, space="PSUM") as psDo,
            ):
                def passC(n0, nn):
                    nc.vector.tensor_mul(u0[:, n0:n0 + nn],
                                         yP[0][:, n0:n0 + nn],
                                         varP[:, n0:n0 + nn])
                    nc.gpsimd.tensor_mul(u1[0:96, n0:n0 + nn],
                                         yP[1][:, n0:n0 + nn],
                                         varP[:, n0:n0 + nn])
                    nc.vector.tensor_mul(u1[96:97, n0:n0 + nn],
                                         muP[0:1, n0:n0 + nn],
                                         varP[0:1, n0:n0 + nn])
                    pdc = psC.tile([112, 512], F32, tag="pdc")
                    nc.tensor.matmul(pdc[:, :nn], s_wdaA[:, 0:112],
                                     u0[:, n0:n0 + nn], start=True, stop=False)
                    nc.tensor.matmul(pdc[:, :nn], s_wdaB[:, 0:112],
                                     u1[:, n0:n0 + nn], start=False, stop=True)
                    pdd = psD2.tile([128, 512], F32, tag="pdd")
                    nc.tensor.matmul(pdd[:, :nn], s_wdaA[:, 112:240],
                                     u0[:, n0:n0 + nn], start=True, stop=False)
                    nc.tensor.matmul(pdd[:, :nn], s_wdaB[:, 112:240],
                                     u1[:, n0:n0 + nn], start=False, stop=True)
                    nc.scalar.activation(out=xaP[:, n0:n0 + nn],
                                         in_=pdc[:, :nn],
                                         func=AF.Tanh, bias=s_bda, scale=1.0)
                    nc.scalar.activation(out=xbP[:, n0:n0 + nn],
                                         in_=pdd[96:128, :nn],
                                         func=AF.Tanh, bias=s_bdb, scale=1.0)
                    nc.scalar.activation(out=xaP[:, n0:n0 + nn],
                                         in_=xaP[:, n0:n0 + nn],
                                         func=AF.Copy, scale=s_sgA)
                    nc.scalar.activation(out=xbP[:, n0:n0 + nn],
                                         in_=xbP[:, n0:n0 + nn],
                                         func=AF.Copy, scale=s_sgB)
                    exw = wpool.tile([72, 512], BF16, tag="exw")
                    nc.scalar.activation(out=exw[:, :nn], in_=pdd[0:72, :nn],
                                         func=AF.Exp, bias=s_blog, scale=1.0)
                    pz = psE.tile([72, 512], F32, tag="pz")
                    nc.tensor.matmul(pz[:, :nn], s_bd6[:], exw[:, :nn],
                                     start=True, stop=True)
                    rz = wpool.tile([72, 512], F32, tag="rz")
                    nc.vector.reciprocal_approx_fast(out=rz[:, :nn],
                                                     in_=pz[:, :nn])
                    nc.vector.tensor_mul(xwP[:, n0:n0 + nn], exw[:, :nn],
                                         rz[:, :nn])

                # ======== phase F: hats, kappa, sampling, w_o ========
                pcolv = [0]
                def S1(q):
                    c0 = q * 128
                    sg = segs[q]
                    nseg = len(sg)
                    st = {}
                    pT = psT.tile([128, 216], F32, tag="pT", name="pT")
                    nc.tensor.transpose(pT[:, 0:112], xaP[:, c0:c0 + 128],
                                        s_idf[:112, :112])
                    nc.tensor.transpose(pT[:, 112:144], xbP[:, c0:c0 + 128],
                                        s_idf[:32, :32])
                    nc.tensor.transpose(pT[:, 144:216], xwP[:, c0:c0 + 128],
                                        s_idf[:72, :72])
                    rm = kpool.tile([128, 216], F32, tag="rm", name="rm")
                    nc.scalar.copy(out=rm[:], in_=pT[:])
                    patch = ppool.tile([KWPAD, nseg * D], BF16, tag="patch",
                                       name="patch")
                    pcol = pcolv[0]
                    nc.sync.dma_start(patch[:],
                                      pblob[:, pcol * D:(pcol + nseg) * D])
                    pcolv[0] += nseg
                    hxy = kpool.tile([128, 2 * HATB], F32, tag="hxy",
                                     name="hxy")
                    for coord in range(2):
                        eng = nc.vector if coord == 0 else nc.gpsimd
                        for l in range(NL):
                            w = WXY[l]
                            out_ap = _ap(hxy[:], coord * HATB + HOFF[l],
                                         [[72, 6], [w, 4], [1, w]])
                            in0 = _ap(rm[:], 8 * l + coord,
                                      [[24, 6], [2, 4], [0, w]])
                            in1 = _ap(s_iota[:], coord * HATB + HOFF[l],
                                      [[72, 6], [w, 4], [1, w]])
                            eng.tensor_sub(out_ap, in0, in1)
                    st['rm'], st['hxy'], st['patch'] = rm, hxy, patch
                    return st

                def S2a(q, st):
                    hs = st['hxy'][:]
                    nc.scalar.activation(out=hs, in_=hs, func=AF.Abs)
                    nc.scalar.activation(out=hs, in_=hs, func=AF.Relu,
                                         bias=1.0, scale=-1.0)

                def S2(q, st):
                    rm, hxy = st['rm'], st['hxy']
                    tail = q >= NCH - 2   # vector drains last chunks
                    for l in range(NL):
                        w = WXY[l]
                        hy_ap = _ap(hxy[:], HATB + HOFF[l],
                                    [[72, 6], [w, 4], [1, w]])
                        wt_ap = _ap(rm[:], 144 + 4 * l,
                                    [[12, 6], [1, 4], [0, w]])
                        eng = nc.gpsimd if (l == 0 and not tail) else nc.vector
                        eng.tensor_mul(hy_ap, hy_ap, wt_ap)
                    kap = kpool.tile([128, 6 * KWIN], BF16, tag="kap",
                                     name="kap")
                    tmp = kpool.tile([128, 6 * 4 * WXY[0] ** 2], F32,
                                     tag="tmp", name="tmp")
                    for l in range(NL):
                        w = WXY[l]
                        for m in range(4):
                            hy = _ap(hxy[:], HATB + HOFF[l] + m * w,
                                     [[72, 6], [1, w], [0, w]])
                            hx = _ap(hxy[:], HOFF[l] + m * w,
                                     [[72, 6], [0, w], [1, w]])
                            t1 = _ap(tmp[:], m * w * w,
                                     [[4 * w * w, 6], [w, w], [1, w]])
                            eng = (nc.gpsimd if m == 3 else nc.vector
                                   ) if tail else (
                                   nc.gpsimd if m % 2 else nc.vector)
                            eng.tensor_mul(t1, hy, hx)
                        t2a = _ap(tmp[:], 0,
                                  [[4 * w * w, 6], [w * w, 2], [w, w], [1, w]])
                        t2b = _ap(tmp[:], 2 * w * w,
                                  [[4 * w * w, 6], [w * w, 2], [w, w], [1, w]])
                        eng = nc.vector if (l == 0 or tail) else nc.gpsimd
                        eng.tensor_add(t2a, t2a, t2b)
                        ksl = _ap(kap[:], LOFF[l], [[KWIN, 6], [w, w], [1, w]])
                        t1a = _ap(tmp[:], 0, [[4 * w * w, 6], [w, w], [1, w]])
                        t1b = _ap(tmp[:], w * w,
                                  [[4 * w * w, 6], [w, w], [1, w]])
                        eng = nc.vector if tail else (
                            nc.gpsimd if l == 0 else nc.vector)
                        eng.tensor_add(ksl, t1a, t1b)
                    st['kap'] = kap

                def S3(q, st):
                    c0 = q * 128
                    sg = segs[q]
                    kap, patch = st['kap'], st['patch']
                    pK = psK.tile([122, 6, 128], BF16, tag="pK", name="pK")
                    for hh in range(H):
                        nc.tensor.transpose(pK[:, hh, :],
                                            kap[:, hh * KWIN:(hh + 1) * KWIN],
                                            s_idb[:])
                    kT = kpool.tile([122, 6, 128], BF16, tag="kT", name="kT")
                    nc.scalar.copy(out=kT[:, 0:3, :], in_=pK[:, 0:3, :])
                    nc.vector.tensor_copy(kT[:, 3:6, :], pK[:, 3:6, :])
                    pXt = psX.tile([96, 8, 128], F32, tag="pXt", name="pXt")
                    pXa = pXt[:, 0:3, :]
                    pXb = pXt[:, 4:7, :]
                    for j, (r, s0, n) in enumerate(sg):
                        nc.tensor.matmul(
                            pXa[:, :, s0:s0 + n],
                            patch[0:KWIN, j * D:j * D + 96],
                            kT[:, 0:3, s0:s0 + n],
                            start=True, stop=True)
                        nc.tensor.matmul(
                            pXb[:, :, s0:s0 + n],
                            patch[0:KWIN, j * D + 96:j * D + 192],
                            kT[:, 3:6, s0:s0 + n],
                            start=True, stop=True)
                    XU = kpool.tile([96, 2, 128], BF16, tag="XU", name="XU")
                    for hh in range(3):
                        base = pXt[32 * hh:32 * hh + 32, 0, :]
                        nc.scalar.copy(
                            out=XU[32 * hh:32 * hh + 32, :, :],
                            in_=_ap(base, hh * 128, [[512, 2], [1, 128]]))
                    pDt = psDo.tile([96, 2, 128], F32, tag="pDt", name="pDt")
                    od = kpool.tile([96, 2, 128], F32, tag="od", name="od")
                    for mc in range(2):
                        nc.tensor.matmul(pDt[:, mc, :],
                                         s_woA[:, mc * 96:(mc + 1) * 96],
                                         XU[:, 0, :], start=True, stop=False)
                        nc.tensor.matmul(pDt[:, mc, :],
                                         s_woB[:, mc * 96:(mc + 1) * 96],
                                         XU[:, 1, :], start=False, stop=False)
                        nc.tensor.matmul(pDt[:, mc, :],
                                         s_wob[:, mc * 96:(mc + 1) * 96],
                                         s_one1[:], start=False, stop=True)
                    nc.scalar.copy(out=od[:], in_=pDt[:])
                    nc.sync.dma_start(
                        bass.AP(tensor=outT[:].tensor, offset=c0,
                                ap=[[SP, 96], [96 * SP, 2], [1, 128]]),
                        od[:])

                # interleave pass C blocks with a 2-stage software skew of
                # phase F (engine queues are in-order; interleaving chunks
                # fills cross-engine handoff bubbles, and starting F right
                # after C(0) overlaps the lead-in)
                nblk = len(NCS)
                blk_end = [(n0 + nn) // 128 for n0, nn in NCS]
                emitted_c = [0]
                def needC(t):
                    while emitted_c[0] < nblk and (
                            0 if emitted_c[0] == 0
                            else blk_end[emitted_c[0] - 1]) < t + 1:
                        j = emitted_c[0]
                        passC(NCS[j][0], NCS[j][1])
                        emitted_c[0] += 1
                sts = [None] * NCH
                for t in range(NCH + 3):
                    if t < NCH:
                        needC(min(t + 1, NCH - 1))
                        sts[t] = S1(t)
                    if 0 <= t - 2 < NCH:
                        S2a(t - 2, sts[t - 2])
                        S2(t - 2, sts[t - 2])
                    if t - 3 >= 0:
                        S3(t - 3, sts[t - 3])
    nc.compile()
    return nc


def _host_prep(inputs, plan):
    h = inputs["h"].astype(np.float32)
    ti = np.asarray(inputs["top_indices"], np.int64)
    qc = inputs["query_coords"].astype(np.float32)
    g = inputs["g"].astype(np.float32)
    maps = [np.asarray(inputs["L2_proj"], np.float32),
            np.asarray(inputs["L3_proj"], np.float32),
            np.asarray(inputs["L4_proj"], np.float32)]
    B, K, R = ti.shape
    cap, bnd, SP, NCH, segs = (plan['cap'], plan['bnd'], plan['SP'],
                               plan['NCH'], plan['segs'])
    order = plan['order']

    consts = {}
    consts["wu"] = np.ascontiguousarray(inputs["w_u_w"].T).astype(ml_dtypes.bfloat16)
    consts["wub"] = inputs["w_u_b"].reshape(D, 1).astype(np.float32)
    # LN fold: z = Wg.(y*rr) - rowsum(Wg).(mu*rr) + (W.b + c)
    gam = inputs["ln_u_g"].astype(np.float32)
    bet = inputs["ln_u_b"].astype(np.float32)
    Wall = np.concatenate([inputs["w_delta_w"], inputs["w_a_w"]], 0)  # [216,192]
    ball = np.concatenate([inputs["w_delta_b"], inputs["w_a_b"]], 0)  # [216]
    Wg = Wall * gam[None, :]
    Wg240 = np.zeros((240, D), np.float32)
    Wg240[0:112] = Wg[0:112]
    Wg240[112:184] = Wg[144:216]
    Wg240[208:240] = Wg[112:144]
    lhs = np.concatenate([Wg240.T, -Wg240.sum(1)[None, :]], 0)  # [193, 240]
    consts["wdaA"] = lhs[0:96].astype(ml_dtypes.bfloat16)
    consts["wdaB"] = lhs[96:193].astype(ml_dtypes.bfloat16)
    biasf = Wall @ bet + ball                              # [216]
    consts["bda"] = biasf[0:112].reshape(112, 1).astype(np.float32)
    consts["bdb"] = biasf[112:144].reshape(32, 1).astype(np.float32)
    consts["blog"] = biasf[144:216].reshape(72, 1).astype(np.float32)
    consts["bd6"] = np.kron(np.eye(H, dtype=np.float32),
                            np.ones((12, 12), np.float32)).astype(ml_dtypes.bfloat16)
    # per-offset-row sigma (rows (h,l,m,c): l = (o//8)%3)
    sv = np.array([SIGMAS[(o // 8) % 3] for o in range(144)], np.float32)
    consts["sgA"] = sv[0:112].reshape(112, 1)
    consts["sgB"] = sv[112:144].reshape(32, 1)
    # iota: (i - clo); device x = sig*tanh, so hat = relu(1-|x - iota|)
    io = np.zeros((128, 2 * HATB + VTAG), np.float32)
    for coord in range(2):
        for l in range(NL):
            w = WXY[l]
            for hh in range(H):
                for m in range(M):
                    st = coord * HATB + HOFF[l] + 72 * hh + w * m
                    io[:, st:st + w] = np.arange(w, dtype=np.float32) - CLO[l]
    consts["iotah"] = io.astype(ml_dtypes.bfloat16)
    consts["onesw"] = np.ones((96, 96), ml_dtypes.bfloat16)
    consts["identf"] = np.eye(128, dtype=np.float32)
    consts["identb"] = np.eye(128, dtype=ml_dtypes.bfloat16)
    woT = np.ascontiguousarray(inputs["w_o_w"].T).astype(np.float32)
    consts["woA"] = woT[0:96].astype(ml_dtypes.bfloat16)
    consts["woB"] = woT[96:192].astype(ml_dtypes.bfloat16)
    consts["wob"] = (inputs["w_o_b"] + inputs["e_deform"].reshape(-1)
                     ).reshape(1, D).astype(ml_dtypes.bfloat16)

    pmaps = []
    for b in range(B):
        pm = []
        for l in range(NL):
            Wl = maps[l].shape[3]
            mp = np.transpose(maps[l][b], (1, 2, 0))
            Hp = 32 * SCALE[l] + WXY[l]
            out = np.zeros((Hp, Hp, D), np.float32)
            out[PADL[l]:PADL[l] + Wl, PADL[l]:PADL[l] + Wl] = mp
            pm.append(out.astype(ml_dtypes.bfloat16))
        pmaps.append(pm)

    freqs = 2.0 ** np.arange(NF, dtype=np.float32)
    cell_of = ti.reshape(B, K * R)

    in_maps, slot_maps = [], []
    for q in range(8):
        b, crow = q // 4, q % 4
        d = dict(consts)
        # rank -> cell (local id in 0..255), patches in per-chunk segment order
        r2c = order[q]
        cells_seq = []
        for ch in range(NCH):
            for (r, s0, n) in segs[ch]:
                cells_seq.append(r2c[r])
        pats_all = np.zeros((KWPAD, len(cells_seq) * D), ml_dtypes.bfloat16)
        for j, cid in enumerate(cells_seq):
            ayc, axc = cid // 32, cid % 32
            col = []
            for l in range(NL):
                w = WXY[l]
                pm = pmaps[b][l]
                r0 = SCALE[l] * 8 * crow
                ys = (r0 + SCALE[l] * ayc) + np.arange(w)
                xs = (SCALE[l] * axc) + np.arange(w)
                pt = pm[ys[:, None], xs[None, :], :]     # [w, w, D]
                col.append(pt.reshape(w * w, D))
            pats_all[:KWIN, j * D:(j + 1) * D] = np.concatenate(col, 0)
        d["pblob"] = pats_all

        # slot -> token
        slot_tok = -np.ones(SP, np.int64)
        cnt = plan['counts'][q]
        for r in range(256):
            cid = r2c[r]
            gcid = crow * 256 + cid
            toks = np.nonzero(cell_of[b] == gcid)[0]
            s0 = int(bnd[r] - cap[r])
            assert len(toks) <= cap[r]
            slot_tok[s0:s0 + len(toks)] = toks
        valid = slot_tok >= 0
        st = np.where(valid, slot_tok, 0)
        k_of = st // R
        cid_of = cell_of[b][st]
        h_s = h[b][k_of] * valid[:, None]
        g_s = g[b][cid_of] * valid[:, None]
        qc_s = qc[b][k_of]
        ax = (cid_of % 32).astype(np.float32)
        ay = (cid_of // 32).astype(np.float32)
        anchor = np.stack([ax * 32 + 16, ay * 32 + 16], -1)
        dp = (anchor - qc_s) / 1024.0
        xf = dp[:, 0:1] * freqs * 2 * np.pi
        yf = dp[:, 1:2] * freqs * 2 * np.pi
        phi = np.concatenate([np.sin(xf), np.cos(xf), np.sin(yf), np.cos(yf)],
                             -1).astype(np.float32) * valid[:, None]
        u_in = np.concatenate([h_s, g_s, phi], -1)
        uT = np.zeros((512, SP), ml_dtypes.bfloat16)
        uT[0:416] = np.ascontiguousarray(u_in.T).astype(ml_dtypes.bfloat16)
        d["uinT"] = uT
        in_maps.append(d)
        slot_maps.append((slot_tok, valid))
    return in_maps, slot_maps


def kernel(**inputs):
    plan = _plan(inputs["top_indices"])
    key = plan['SP'], tuple(plan['cap'].tolist())
    if _CACHE.get("key") != key:
        _CACHE["nc"] = _build_module(plan)
        _CACHE["key"] = key
    nc = _CACHE["nc"]
    in_maps, slot_maps = _host_prep(inputs, plan)
    res = run_bass_kernel_spmd(nc, in_maps, core_ids=list(range(8)),
                               **_CACHE.get("run_kwargs", {}))
    _CACHE["last"] = res
    B, K, R = inputs["top_indices"].shape
    out = np.zeros((B, K * R, D), np.float32)
    for q in range(8):
        b = q // 4
        oT = np.asarray(res.results[q]["outT"], np.float32)
        slot_tok, valid = slot_maps[q]
        out[b, slot_tok[valid]] = oT.T[valid]
    return out.reshape(B, K, R, D)


# revision 72
# speedup vs baseline: 1.0148x; 1.0112x over previous
"""Trainium2 Bass kernel for nn_DeformableRead (deformable attention read).

8 NeuronCores SPMD: core q -> batch q//4, anchor-cell rows 8*(q%4)..+8 (256
cells). Tokens routed to the core owning their anchor cell (host permutation).
Sample points live in fixed windows around each anchor cell (9x9/5x5/4x4 at
L2/L3/L4); bilinear sampling over a window is a dense 122-tap PE contraction
with separable hat weights relu(1-|x-i|) -- gather-free.

v3 (346us -> ~258us): patch blob padded to 128 partitions so each chunk DMA
spreads over all 16 SDMA engines (HWDGE splits a transfer across
gcd(outer_dim,16) engines; 122 rows -> only 2 engines at 26 GB/s = 253us DMA
critical path). Output DMA issued from sync engine (scalar is busy in phase
F). Pass C interleaved with phase F chunks (2-stage software skew) so the
hat/sampling pipeline starts after the first 512-slot block instead of after
all of pass C. Pass C writes tanh directly (clo folded into the iota table,
sigma applied on scalar as a per-partition-scale Copy activation). XU psum
evacuation merged 6->3 copies, od 2->1. Pass A square on gpsimd (idle in
lead-in). Last two chunks' hat/kappa ops biased to vector (drains ~10us
earlier than gpsimd, shortening the tail).
Measured engine quirks honored: f32 1x TT everywhere (bf16 strided
TT and 2-op tensor_scalar chains hit slow paths; gpsimd tensor_scalar is
~10x slower than DVE; matmul-transpose ignores its rhs values so no diag
scaling; DMA transpose from SBUF breaks).
Host does layout only: sharding, slot permutation, patch extraction, bf16
casts, fourier features of raw coords, constants. Device does all heavy math.
"""

import numpy as np
import ml_dtypes

import concourse.bass as bass
import concourse.bacc as bacc
import concourse.tile as tile
from concourse import mybir
from concourse.bass_utils import run_bass_kernel_spmd

D, H, NL, M = 192, 6, 3, 4
NF = 8
SIGMAS = (4.0, 2.0, 1.0)
WXY = (9, 5, 4)
CLO = (4.0, 2.0, 1.5)
PADL = (2, 1, 1)
SCALE = (4, 2, 1)
KWIN = sum(w * w for w in WXY)  # 122
LOFF = (0, WXY[0] ** 2, WXY[0] ** 2 + WXY[1] ** 2)
HATW = sum(4 * w for w in WXY)  # 72 per head per coord
HOFF = (0, 36, 56)
HATB = 6 * HATW  # 432 per coord
BF16 = mybir.dt.bfloat16
F32 = mybir.dt.float32

_CACHE = {}
VTAG = 15  # bump to invalidate terminal-side NEFF cache (shape-keyed)
KWPAD = 128  # patch partition dim padded 122->128: DMA splits across
             # gcd(outer_dim, 16) engines, so 122 -> only 2 engines


def _ap(base, free_off, dims):
    """Custom AP: base tile slice (sets partition range), explicit free dims."""
    return bass.AP(tensor=base.tensor, offset=base.offset + free_off,
                   ap=[base.ap[0]] + [list(d) for d in dims])


def _plan(top_indices):
    """Shared (cross-core) packing plan from top_indices."""
    ti = np.asarray(top_indices, np.int64)
    B, K, R = ti.shape
    counts = np.zeros((8, 256), np.int64)
    for q in range(8):
        b, crow = q // 4, q % 4
        cells = ti[b].reshape(-1)
        sel = cells[(cells >= crow * 256) & (cells < (crow + 1) * 256)] - crow * 256
        counts[q] = np.bincount(sel, minlength=256)
    order = np.argsort(counts, axis=1, kind='stable')  # per core: rank -> cell
    srt = np.sort(counts, axis=1)         # ascending: many-seg chunks first
    cap = srt.max(0)                      # capacity per rank
    cap = np.maximum(cap, 1)              # every rank owns >= 1 slot
    bnd = np.cumsum(cap)
    S2 = int(bnd[-1])
    NCH = (S2 + 127) // 128
    SP = NCH * 128
    # segments per chunk: (rank, s0_in_chunk, n)
    segs = [[] for _ in range(NCH)]
    for r in range(256):
        s0, s1 = int(bnd[r] - cap[r]), int(bnd[r])
        for ch in range(s0 // 128, (s1 - 1) // 128 + 1):
            a = max(s0, ch * 128)
            b_ = min(s1, (ch + 1) * 128)
            segs[ch].append((r, a - ch * 128, b_ - a))
    # extend final segment to cover padding tail
    if S2 < SP:
        r, a, n = segs[-1][-1]
        segs[-1][-1] = (r, a, n + SP - S2)
    return dict(counts=counts, order=order, cap=cap, bnd=bnd, S2=S2,
                SP=SP, NCH=NCH, segs=segs)


def _build_module(plan):
    SP, NCH, segs = plan['SP'], plan['NCH'], plan['segs']
    nsegtot = sum(len(s) for s in segs)
    nc = bacc.Bacc("TRN2", target_bir_lowering=False, debug=False)
    dt = nc.dram_tensor
    uinT = dt("uinT", [512, SP], BF16, kind="ExternalInput")
    pblob = dt("pblob", [KWPAD, nsegtot * D], BF16, kind="ExternalInput")
    wu = dt("wu", [416, D], BF16, kind="ExternalInput")
    wub = dt("wub", [D, 1], F32, kind="ExternalInput")
    wdaA = dt("wdaA", [96, 240], BF16, kind="ExternalInput")
    wdaB = dt("wdaB", [97, 240], BF16, kind="ExternalInput")
    bda = dt("bda", [112, 1], F32, kind="ExternalInput")
    bdb = dt("bdb", [32, 1], F32, kind="ExternalInput")
    blog = dt("blog", [72, 1], F32, kind="ExternalInput")
    bd6 = dt("bd6", [72, 72], BF16, kind="ExternalInput")
    sgA = dt("sgA", [112, 1], F32, kind="ExternalInput")
    sgB = dt("sgB", [32, 1], F32, kind="ExternalInput")
    iotah = dt("iotah", [128, 2 * HATB + VTAG], BF16, kind="ExternalInput")
    onesw = dt("onesw", [96, 96], BF16, kind="ExternalInput")
    identf = dt("identf", [128, 128], F32, kind="ExternalInput")
    identb = dt("identb", [128, 128], BF16, kind="ExternalInput")
    woA = dt("woA", [96, D], BF16, kind="ExternalInput")
    woB = dt("woB", [96, D], BF16, kind="ExternalInput")
    wob = dt("wob", [1, D], BF16, kind="ExternalInput")
    outT = dt("outT", [D, SP], F32, kind="ExternalOutput")

    NCS = [(i * 512, min(512, SP - i * 512)) for i in range((SP + 511) // 512)]
    AF = mybir.ActivationFunctionType
    OP = mybir.AluOpType

    with tile.TileContext(nc) as tc:
        with (
            tc.tile_pool(name="const", bufs=1) as cpool,
            tc.tile_pool(name="big", bufs=1) as bpool,
        ):
            _sbn = [0]
            def sb(t_ap, shape, dtype):
                _sbn[0] += 1
                nm = f"cst{_sbn[0]}"
                x = cpool.tile(shape, dtype, tag=nm, name=nm)
                nc.scalar.dma_start(x[:], t_ap)
                return x

            s_wu = []
            for kc in range(4):
                k0, k1 = kc * 128, min((kc + 1) * 128, 416)
                s_wu.append(sb(wu[k0:k1, :], [k1 - k0, D], BF16))
            s_wub = [sb(wub[0:96, :], [96, 1], F32), sb(wub[96:192, :], [96, 1], F32)]
            s_wdaA = sb(wdaA[:], [96, 240], BF16)
            s_wdaB = sb(wdaB[:], [97, 240], BF16)
            s_bda = sb(bda[:], [112, 1], F32)
            s_bdb = sb(bdb[:], [32, 1], F32)
            s_blog = sb(blog[:], [72, 1], F32)
            s_bd6 = sb(bd6[:], [72, 72], BF16)
            s_sgA = sb(sgA[:], [112, 1], F32)
            s_sgB = sb(sgB[:], [32, 1], F32)
            s_iota = sb(iotah[0:128, 0:2 * HATB], [128, 2 * HATB], BF16)
            s_ones = sb(onesw[:], [96, 96], BF16)
            s_idf = sb(identf[:], [128, 128], F32)
            s_idb = sb(identb[:], [128, 128], BF16)
            s_woA = sb(woA[:], [96, D], BF16)
            s_woB = sb(woB[:], [96, D], BF16)
            s_wob = sb(wob[:], [1, D], BF16)
            s_eps = cpool.tile([96, 1], F32, name="s_eps")
            nc.vector.memset(s_eps[:], 1e-5)
            s_one1 = cpool.tile([1, 128], BF16, name="s_one1")
            nc.vector.memset(s_one1[:], 1.0)

            # persistent activations
            yP = [bpool.tile([96, SP], BF16, tag="yP0", name="yP0"),
                  bpool.tile([96, SP], BF16, tag="yP1", name="yP1")]
            muP = bpool.tile([96, SP], F32, tag="muP")
            varP = bpool.tile([96, SP], F32, tag="varP")
            u0 = bpool.tile([96, SP], BF16, tag="u0", name="u0")
            u1 = bpool.tile([97, SP], BF16, tag="u1", name="u1")
            xaP = bpool.tile([112, SP], F32, tag="xaP")
            xbP = bpool.tile([32, SP], F32, tag="xbP")
            xwP = bpool.tile([72, SP], F32, tag="xwP")

            # ======== pass A: u matmul, gelu, stats  (gelu act table) ========
            with (
                tc.tile_pool(name="ucp", bufs=3) as ucpool,
                tc.tile_pool(name="wkA", bufs=2) as wpool,
                tc.tile_pool(name="psA", bufs=2, space="PSUM") as psA,
                tc.tile_pool(name="psB", bufs=2, space="PSUM") as psB,
            ):
                for n0, nn in NCS:
                    uc = ucpool.tile([128, 4, 512], BF16, tag="uc")
                    nc.sync.dma_start(
                        uc[:, :, :nn],
                        bass.AP(tensor=uinT[:].tensor, offset=n0,
                                ap=[[SP, 128], [128 * SP, 4], [1, nn]]))
                    pu = psA.tile([96, 2, 512], F32, tag="pu")
                    for mc in range(2):
                        for kc in range(4):
                            kk = min(128, 416 - kc * 128)
                            nc.tensor.matmul(
                                pu[:, mc, :nn],
                                s_wu[kc][:, mc * 96:(mc + 1) * 96],
                                uc[:kk, kc, :nn],
                                start=(kc == 0), stop=(kc == 3))
                        nc.scalar.activation(
                            out=yP[mc][:, n0:n0 + nn], in_=pu[:, mc, :nn],
                            func=AF.Gelu, bias=s_wub[mc], scale=1.0)
                    y2 = wpool.tile([96, 2, 512], BF16, tag="y2")
                    for mc in range(2):
                        nc.gpsimd.tensor_mul(
                            y2[:, mc, :nn], yP[mc][:, n0:n0 + nn],
                            yP[mc][:, n0:n0 + nn])
                    pst = psB.tile([96, 2, 512], F32, tag="pst")
                    nc.tensor.matmul(pst[:, 0, :nn], s_ones[:],
                                     yP[0][:, n0:n0 + nn], start=True, stop=False)
                    nc.tensor.matmul(pst[:, 0, :nn], s_ones[:],
                                     yP[1][:, n0:n0 + nn], start=False, stop=True)
                    nc.tensor.matmul(pst[:, 1, :nn], s_ones[:],
                                     y2[:, 0, :nn], start=True, stop=False)
                    nc.tensor.matmul(pst[:, 1, :nn], s_ones[:],
                                     y2[:, 1, :nn], start=False, stop=True)
                    nc.vector.tensor_scalar_mul(
                        out=muP[:, n0:n0 + nn], in0=pst[:, 0, :nn],
                        scalar1=1.0 / D)
                    musq = wpool.tile([96, 512], F32, tag="musq")
                    nc.gpsimd.tensor_mul(musq[:, :nn], muP[:, n0:n0 + nn],
                                         muP[:, n0:n0 + nn])
                    nc.vector.scalar_tensor_tensor(
                        out=varP[:, n0:n0 + nn], in0=pst[:, 1, :nn],
                        scalar=1.0 / D, in1=musq[:, :nn],
                        op0=OP.mult, op1=OP.subtract)

            # ======== pass B: rr = 1/sqrt(var+eps)  (sqrt act table) ========
            with tc.tile_pool(name="wkB", bufs=2) as wpool:
                for n0, nn in NCS:
                    sd = wpool.tile([96, 512], F32, tag="sd")
                    nc.scalar.activation(out=sd[:, :nn],
                                         in_=varP[:, n0:n0 + nn],
                                         func=AF.Sqrt, bias=s_eps, scale=1.0)
                    nc.vector.reciprocal_approx_fast(
                        out=varP[:, n0:n0 + nn], in_=sd[:, :nn])

            # ======== pass C (per 512 block) interleaved with phase F ========
            with (
                tc.tile_pool(name="wkC", bufs=2) as wpool,
                tc.tile_pool(name="psC", bufs=1, space="PSUM") as psC,
                tc.tile_pool(name="psD2", bufs=1, space="PSUM") as psD2,
                tc.tile_pool(name="psE", bufs=1, space="PSUM") as psE,
                tc.tile_pool(name="kw", bufs=4) as kpool,
                tc.tile_pool(name="pp", bufs=6) as ppool,
                tc.tile_pool(name="psT", bufs=1, space="PSUM") as psT,
                tc.tile_pool(name="psK", bufs=1, space="PSUM") as psK,
                tc.tile_pool(name="psX", bufs=1, space="PSUM") as psX,
                tc.tile_pool(name="psDo", bufs=1, space="PSUM") as psDo,
            ):
                def passC(n0, nn):
                    nc.vector.tensor_mul(u0[:, n0:n0 + nn],
                                         yP[0][:, n0:n0 + nn],
                                         varP[:, n0:n0 + nn])
                    nc.gpsimd.tensor_mul(u1[0:96, n0:n0 + nn],
                                         yP[1][:, n0:n0 + nn],
                                         varP[:, n0:n0 + nn])
                    nc.vector.tensor_mul(u1[96:97, n0:n0 + nn],
                                         muP[0:1, n0:n0 + nn],
                                         varP[0:1, n0:n0 + nn])
                    pdc = psC.tile([112, 512], F32, tag="pdc")
                    nc.tensor.matmul(pdc[:, :nn], s_wdaA[:, 0:112],
                                     u0[:, n0:n0 + nn], start=True, stop=False)
                    nc.tensor.matmul(pdc[:, :nn], s_wdaB[:, 0:112],
                                     u1[:, n0:n0 + nn], start=False, stop=True)
                    pdd = psD2.tile([128, 512], F32, tag="pdd")
                    nc.tensor.matmul(pdd[:, :nn], s_wdaA[:, 112:240],
                                     u0[:, n0:n0 + nn], start=True, stop=False)
                    nc.tensor.matmul(pdd[:, :nn], s_wdaB[:, 112:240],
                                     u1[:, n0:n0 + nn], start=False, stop=True)
                    nc.scalar.activation(out=xaP[:, n0:n0 + nn],
                                         in_=pdc[:, :nn],
                                         func=AF.Tanh, bias=s_bda, scale=1.0)
                    nc.scalar.activation(out=xbP[:, n0:n0 + nn],
                                         in_=pdd[96:128, :nn],
                                         func=AF.Tanh, bias=s_bdb, scale=1.0)
                    nc.scalar.activation(out=xaP[:, n0:n0 + nn],
                                         in_=xaP[:, n0:n0 + nn],
                                         func=AF.Copy, scale=s_sgA)
                    nc.scalar.activation(out=xbP[:, n0:n0 + nn],
                                         in_=xbP[:, n0:n0 + nn],
                                         func=AF.Copy, scale=s_sgB)
                    exw = wpool.tile([72, 512], BF16, tag="exw")
                    nc.scalar.activation(out=exw[:, :nn], in_=pdd[0:72, :nn],
                                         func=AF.Exp, bias=s_blog, scale=1.0)
                    pz = psE.tile([72, 512], F32, tag="pz")
                    nc.tensor.matmul(pz[:, :nn], s_bd6[:], exw[:, :nn],
                                     start=True, stop=True)
                    rz = wpool.tile([72, 512], F32, tag="rz")
                    nc.vector.reciprocal_approx_fast(out=rz[:, :nn],
                                                     in_=pz[:, :nn])
                    nc.vector.tensor_mul(xwP[:, n0:n0 + nn], exw[:, :nn],
                                         rz[:, :nn])

                # ======== phase F: hats, kappa, sampling, w_o ========
                pcolv = [0]
                def S1(q):
                    c0 = q * 128
                    sg = segs[q]
                    nseg = len(sg)
                    st = {}
                    pT = psT.tile([128, 216], F32, tag="pT", name="pT")
                    nc.tensor.transpose(pT[:, 0:112], xaP[:, c0:c0 + 128],
                                        s_idf[:112, :112])
                    nc.tensor.transpose(pT[:, 112:144], xbP[:, c0:c0 + 128],
                                        s_idf[:32, :32])
                    nc.tensor.transpose(pT[:, 144:216], xwP[:, c0:c0 + 128],
                                        s_idf[:72, :72])
                    rm = kpool.tile([128, 216], F32, tag="rm", name="rm")
                    nc.scalar.copy(out=rm[:], in_=pT[:])
                    patch = ppool.tile([KWPAD, nseg * D], BF16, tag="patch",
                                       name="patch")
                    pcol = pcolv[0]
                    nc.sync.dma_start(patch[:],
                                      pblob[:, pcol * D:(pcol + nseg) * D])
                    pcolv[0] += nseg
                    hxy = kpool.tile([128, 2 * HATB], F32, tag="hxy",
                                     name="hxy")
                    for coord in range(2):
                        eng = nc.vector if coord == 0 else nc.gpsimd
                        for l in range(NL):
                            w = WXY[l]
                            out_ap = _ap(hxy[:], coord * HATB + HOFF[l],
                                         [[72, 6], [w, 4], [1, w]])
                            in0 = _ap(rm[:], 8 * l + coord,
                                      [[24, 6], [2, 4], [0, w]])
                            in1 = _ap(s_iota[:], coord * HATB + HOFF[l],
                                      [[72, 6], [w, 4], [1, w]])
                            eng.tensor_sub(out_ap, in0, in1)
                    st['rm'], st['hxy'], st['patch'] = rm, hxy, patch
                    return st

                def S2a(q, st):
                    hs = st['hxy'][:]
                    nc.scalar.activation(out=hs, in_=hs, func=AF.Abs)
                    nc.scalar.activation(out=hs, in_=hs, func=AF.Relu,
                                         bias=1.0, scale=-1.0)

                def S2(q, st):
                    rm, hxy = st['rm'], st['hxy']
                    tail = q >= NCH - 2   # vector drains last chunks
                    for l in range(NL):
                        w = WXY[l]
                        hy_ap = _ap(hxy[:], HATB + HOFF[l],
                                    [[72, 6], [w, 4], [1, w]])
                        wt_ap = _ap(rm[:], 144 + 4 * l,
                                    [[12, 6], [1, 4], [0, w]])
                        eng = nc.gpsimd if (l == 0 and not tail) else nc.vector
                        eng.tensor_mul(hy_ap, hy_ap, wt_ap)
                    kap = kpool.tile([128, 6 * KWIN], BF16, tag="kap",
                                     name="kap")
                    tmp = kpool.tile([128, 6 * 4 * WXY[0] ** 2], F32,
                                     tag="tmp", name="tmp")
                    for l in range(NL):
                        w = WXY[l]
                        for m in range(4):
                            hy = _ap(hxy[:], HATB + HOFF[l] + m * w,
                                     [[72, 6], [1, w], [0, w]])
                            hx = _ap(hxy[:], HOFF[l] + m * w,
                                     [[72, 6], [0, w], [1, w]])
                            t1 = _ap(tmp[:], m * w * w,
                                     [[4 * w * w, 6], [w, w], [1, w]])
                            eng = (nc.gpsimd if m == 3 else nc.vector
                                   ) if tail else (
                                   nc.gpsimd if m % 2 else nc.vector)
                            eng.tensor_mul(t1, hy, hx)
                        t2a = _ap(tmp[:], 0,
                                  [[4 * w * w, 6], [w * w, 2], [w, w], [1, w]])
                        t2b = _ap(tmp[:], 2 * w * w,
                                  [[4 * w * w, 6], [w * w, 2], [w, w], [1, w]])
                        eng = nc.vector if (l == 0 or tail) else nc.gpsimd
                        eng.tensor_add(t2a, t2a, t2b)
                        ksl = _ap(kap[:], LOFF[l], [[KWIN, 6], [w, w], [1, w]])
                        t1a = _ap(tmp[:], 0, [[4 * w * w, 6], [w, w], [1, w]])
                        t1b = _ap(tmp[:], w * w,
                                  [[4 * w * w, 6], [w, w], [1, w]])
                        eng = nc.vector if tail else (
                            nc.gpsimd if l == 0 else nc.vector)
                        eng.tensor_add(ksl, t1a, t1b)
                    st['kap'] = kap

                def S3(q, st):
                    c0 = q * 128
                    sg = segs[q]
                    kap, patch = st['kap'], st['patch']
                    pK = psK.tile([122, 6, 128], BF16, tag="pK", name="pK")
                    for hh in range(H):
                        nc.tensor.transpose(pK[:, hh, :],
                                            kap[:, hh * KWIN:(hh + 1) * KWIN],
                                            s_idb[:])
                    kT = kpool.tile([122, 6, 128], BF16, tag="kT", name="kT")
                    nc.scalar.copy(out=kT[:, 0:3, :], in_=pK[:, 0:3, :])
                    nc.vector.tensor_copy(kT[:, 3:6, :], pK[:, 3:6, :])
                    pXt = psX.tile([96, 8, 128], F32, tag="pXt", name="pXt")
                    pXa = pXt[:, 0:3, :]
                    pXb = pXt[:, 4:7, :]
                    for j, (r, s0, n) in enumerate(sg):
                        nc.tensor.matmul(
                            pXa[:, :, s0:s0 + n],
                            patch[0:KWIN, j * D:j * D + 96],
                            kT[:, 0:3, s0:s0 + n],
                            start=True, stop=True)
                        nc.tensor.matmul(
                            pXb[:, :, s0:s0 + n],
                            patch[0:KWIN, j * D + 96:j * D + 192],
                            kT[:, 3:6, s0:s0 + n],
                            start=True, stop=True)
                    XU = kpool.tile([96, 2, 128], BF16, tag="XU", name="XU")
                    for hh in range(3):
                        base = pXt[32 * hh:32 * hh + 32, 0, :]
                        nc.scalar.copy(
                            out=XU[32 * hh:32 * hh + 32, :, :],
                            in_=_ap(base, hh * 128, [[512, 2], [1, 128]]))
                    pDt = psDo.tile([96, 2, 128], F32, tag="pDt", name="pDt")
                    od = kpool.tile([96, 2, 128], F32, tag="od", name="od")
                    for mc in range(2):
                        nc.tensor.matmul(pDt[:, mc, :],
                                         s_woA[:, mc * 96:(mc + 1) * 96],
                                         XU[:, 0, :], start=True, stop=False)
                        nc.tensor.matmul(pDt[:, mc, :],
                                         s_woB[:, mc * 96:(mc + 1) * 96],
                                         XU[:, 1, :], start=False, stop=False)
                        nc.tensor.matmul(pDt[:, mc, :],
                                         s_wob[:, mc * 96:(mc + 1) * 96],
                                         s_one1[:], start=False, stop=True)
                    nc.scalar.copy(out=od[:], in_=pDt[:])
                    nc.sync.dma_start(
                        bass.AP(tensor=outT[:].tensor, offset=c0,
                                ap=[[SP, 96], [96 * SP, 2], [1, 128]]),
                        od[:])

                # interleave pass C blocks with a 2-stage software skew of
                # phase F (engine queues are in-order; interleaving chunks
                # fills cross-engine handoff bubbles, and starting F right
                # after C(0) overlaps the lead-in)
                nblk = len(NCS)
                blk_end = [(n0 + nn) // 128 for n0, nn in NCS]
                emitted_c = [0]
                def needC(t):
                    while emitted_c[0] < nblk and (
                            0 if emitted_c[0] == 0
                            else blk_end[emitted_c[0] - 1]) < t + 1:
                        j = emitted_c[0]
                        passC(NCS[j][0], NCS[j][1])
                        emitted_c[0] += 1
                sts = [None] * NCH
                for t in range(NCH + 3):
                    if 0 <= t - 2 < NCH:
                        S2a(t - 2, sts[t - 2])
                    if t < NCH:
                        needC(min(t + 1, NCH - 1))
                        sts[t] = S1(t)
                    if 0 <= t - 2 < NCH:
                        S2(t - 2, sts[t - 2])
                    if t - 3 >= 0:
                        S3(t - 3, sts[t - 3])
    nc.compile()
    return nc


def _host_prep(inputs, plan):
    h = inputs["h"].astype(np.float32)
    ti = np.asarray(inputs["top_indices"], np.int64)
    qc = inputs["query_coords"].astype(np.float32)
    g = inputs["g"].astype(np.float32)
    maps = [np.asarray(inputs["L2_proj"], np.float32),
            np.asarray(inputs["L3_proj"], np.float32),
            np.asarray(inputs["L4_proj"], np.float32)]
    B, K, R = ti.shape
    cap, bnd, SP, NCH, segs = (plan['cap'], plan['bnd'], plan['SP'],
                               plan['NCH'], plan['segs'])
    order = plan['order']

    consts = {}
    consts["wu"] = np.ascontiguousarray(inputs["w_u_w"].T).astype(ml_dtypes.bfloat16)
    consts["wub"] = inputs["w_u_b"].reshape(D, 1).astype(np.float32)
    # LN fold: z = Wg.(y*rr) - rowsum(Wg).(mu*rr) + (W.b + c)
    gam = inputs["ln_u_g"].astype(np.float32)
    bet = inputs["ln_u_b"].astype(np.float32)
    Wall = np.concatenate([inputs["w_delta_w"], inputs["w_a_w"]], 0)  # [216,192]
    ball = np.concatenate([inputs["w_delta_b"], inputs["w_a_b"]], 0)  # [216]
    Wg = Wall * gam[None, :]
    Wg240 = np.zeros((240, D), np.float32)
    Wg240[0:112] = Wg[0:112]
    Wg240[112:184] = Wg[144:216]
    Wg240[208:240] = Wg[112:144]
    lhs = np.concatenate([Wg240.T, -Wg240.sum(1)[None, :]], 0)  # [193, 240]
    consts["wdaA"] = lhs[0:96].astype(ml_dtypes.bfloat16)
    consts["wdaB"] = lhs[96:193].astype(ml_dtypes.bfloat16)
    biasf = Wall @ bet + ball                              # [216]
    consts["bda"] = biasf[0:112].reshape(112, 1).astype(np.float32)
    consts["bdb"] = biasf[112:144].reshape(32, 1).astype(np.float32)
    consts["blog"] = biasf[144:216].reshape(72, 1).astype(np.float32)
    consts["bd6"] = np.kron(np.eye(H, dtype=np.float32),
                            np.ones((12, 12), np.float32)).astype(ml_dtypes.bfloat16)
    # per-offset-row sigma (rows (h,l,m,c): l = (o//8)%3)
    sv = np.array([SIGMAS[(o // 8) % 3] for o in range(144)], np.float32)
    consts["sgA"] = sv[0:112].reshape(112, 1)
    consts["sgB"] = sv[112:144].reshape(32, 1)
    # iota: (i - clo); device x = sig*tanh, so hat = relu(1-|x - iota|)
    io = np.zeros((128, 2 * HATB + VTAG), np.float32)
    for coord in range(2):
        for l in range(NL):
            w = WXY[l]
            for hh in range(H):
                for m in range(M):
                    st = coord * HATB + HOFF[l] + 72 * hh + w * m
                    io[:, st:st + w] = np.arange(w, dtype=np.float32) - CLO[l]
    consts["iotah"] = io.astype(ml_dtypes.bfloat16)
    consts["onesw"] = np.ones((96, 96), ml_dtypes.bfloat16)
    consts["identf"] = np.eye(128, dtype=np.float32)
    consts["identb"] = np.eye(128, dtype=ml_dtypes.bfloat16)
    woT = np.ascontiguousarray(inputs["w_o_w"].T).astype(np.float32)
    consts["woA"] = woT[0:96].astype(ml_dtypes.bfloat16)
    consts["woB"] = woT[96:192].astype(ml_dtypes.bfloat16)
    consts["wob"] = (inputs["w_o_b"] + inputs["e_deform"].reshape(-1)
                     ).reshape(1, D).astype(ml_dtypes.bfloat16)

    pmaps = []
    for b in range(B):
        pm = []
        for l in range(NL):
            Wl = maps[l].shape[3]
            mp = np.transpose(maps[l][b], (1, 2, 0))
            Hp = 32 * SCALE[l] + WXY[l]
            out = np.zeros((Hp, Hp, D), np.float32)
            out[PADL[l]:PADL[l] + Wl, PADL[l]:PADL[l] + Wl] = mp
            pm.append(out.astype(ml_dtypes.bfloat16))
        pmaps.append(pm)

    freqs = 2.0 ** np.arange(NF, dtype=np.float32)
    cell_of = ti.reshape(B, K * R)

    in_maps, slot_maps = [], []
    for q in range(8):
        b, crow = q // 4, q % 4
        d = dict(consts)
        # rank -> cell (local id in 0..255), patches in per-chunk segment order
        r2c = order[q]
        cells_seq = []
        for ch in range(NCH):
            for (r, s0, n) in segs[ch]:
                cells_seq.append(r2c[r])
        pats_all = np.zeros((KWPAD, len(cells_seq) * D), ml_dtypes.bfloat16)
        for j, cid in enumerate(cells_seq):
            ayc, axc = cid // 32, cid % 32
            col = []
            for l in range(NL):
                w = WXY[l]
                pm = pmaps[b][l]
                r0 = SCALE[l] * 8 * crow
                ys = (r0 + SCALE[l] * ayc) + np.arange(w)
                xs = (SCALE[l] * axc) + np.arange(w)
                pt = pm[ys[:, None], xs[None, :], :]     # [w, w, D]
                col.append(pt.reshape(w * w, D))
            pats_all[:KWIN, j * D:(j + 1) * D] = np.concatenate(col, 0)
        d["pblob"] = pats_all

        # slot -> token
        slot_tok = -np.ones(SP, np.int64)
        cnt = plan['counts'][q]
        for r in range(256):
            cid = r2c[r]
            gcid = crow * 256 + cid
            toks = np.nonzero(cell_of[b] == gcid)[0]
            s0 = int(bnd[r] - cap[r])
            assert len(toks) <= cap[r]
            slot_tok[s0:s0 + len(toks)] = toks
        valid = slot_tok >= 0
        st = np.where(valid, slot_tok, 0)
        k_of = st // R
        cid_of = cell_of[b][st]
        h_s = h[b][k_of] * valid[:, None]
        g_s = g[b][cid_of] * valid[:, None]
        qc_s = qc[b][k_of]
        ax = (cid_of % 32).astype(np.float32)
        ay = (cid_of // 32).astype(np.float32)
        anchor = np.stack([ax * 32 + 16, ay * 32 + 16], -1)
        dp = (anchor - qc_s) / 1024.0
        xf = dp[:, 0:1] * freqs * 2 * np.pi
        yf = dp[:, 1:2] * freqs * 2 * np.pi
        phi = np.concatenate([np.sin(xf), np.cos(xf), np.sin(yf), np.cos(yf)],
                             -1).astype(np.float32) * valid[:, None]
        u_in = np.concatenate([h_s, g_s, phi], -1)
        uT = np.zeros((512, SP), ml_dtypes.bfloat16)
        uT[0:416] = np.ascontiguousarray(u_in.T).astype(ml_dtypes.bfloat16)
        d["uinT"] = uT
        in_maps.append(d)
        slot_maps.append((slot_tok, valid))
    return in_maps, slot_maps


def kernel(**inputs):
    plan = _plan(inputs["top_indices"])
    key = plan['SP'], tuple(plan['cap'].tolist())
    if _CACHE.get("key") != key:
        _CACHE["nc"] = _build_module(plan)
        _CACHE["key"] = key
    nc = _CACHE["nc"]
    in_maps, slot_maps = _host_prep(inputs, plan)
    res = run_bass_kernel_spmd(nc, in_maps, core_ids=list(range(8)),
                               **_CACHE.get("run_kwargs", {}))
    _CACHE["last"] = res
    B, K, R = inputs["top_indices"].shape
    out = np.zeros((B, K * R, D), np.float32)
    for q in range(8):
        b = q // 4
        oT = np.asarray(res.results[q]["outT"], np.float32)
        slot_tok, valid = slot_maps[q]
        out[b, slot_tok[valid]] = oT.T[valid]
    return out.reshape(B, K, R, D)


# revision 73
# speedup vs baseline: 1.0285x; 1.0135x over previous
"""Trainium2 Bass kernel for nn_DeformableRead (deformable attention read).

8 NeuronCores SPMD: core q -> batch q//4, anchor-cell rows 8*(q%4)..+8 (256
cells). Tokens routed to the core owning their anchor cell (host permutation).
Sample points live in fixed windows around each anchor cell (9x9/5x5/4x4 at
L2/L3/L4); bilinear sampling over a window is a dense 122-tap PE contraction
with separable hat weights relu(1-|x-i|) -- gather-free.

v3 (346us -> ~258us): patch blob padded to 128 partitions so each chunk DMA
spreads over all 16 SDMA engines (HWDGE splits a transfer across
gcd(outer_dim,16) engines; 122 rows -> only 2 engines at 26 GB/s = 253us DMA
critical path). Output DMA issued from sync engine (scalar is busy in phase
F). Pass C interleaved with phase F chunks (2-stage software skew) so the
hat/sampling pipeline starts after the first 512-slot block instead of after
all of pass C. Pass C writes tanh directly (clo folded into the iota table,
sigma applied on scalar as a per-partition-scale Copy activation). XU psum
evacuation merged 6->3 copies, od 2->1. Pass A square on gpsimd (idle in
lead-in). Last two chunks' hat/kappa ops biased to vector (drains ~10us
earlier than gpsimd, shortening the tail).
Measured engine quirks honored: f32 1x TT everywhere (bf16 strided
TT and 2-op tensor_scalar chains hit slow paths; gpsimd tensor_scalar is
~10x slower than DVE; matmul-transpose ignores its rhs values so no diag
scaling; DMA transpose from SBUF breaks).
Host does layout only: sharding, slot permutation, patch extraction, bf16
casts, fourier features of raw coords, constants. Device does all heavy math.
"""

import numpy as np
import ml_dtypes

import concourse.bass as bass
import concourse.bacc as bacc
import concourse.tile as tile
from concourse import mybir
from concourse.bass_utils import run_bass_kernel_spmd

D, H, NL, M = 192, 6, 3, 4
NF = 8
SIGMAS = (4.0, 2.0, 1.0)
WXY = (9, 5, 4)
CLO = (4.0, 2.0, 1.5)
PADL = (2, 1, 1)
SCALE = (4, 2, 1)
KWIN = sum(w * w for w in WXY)  # 122
LOFF = (0, WXY[0] ** 2, WXY[0] ** 2 + WXY[1] ** 2)
HATW = sum(4 * w for w in WXY)  # 72 per head per coord
HOFF = (0, 36, 56)
HATB = 6 * HATW  # 432 per coord
BF16 = mybir.dt.bfloat16
F32 = mybir.dt.float32

_CACHE = {}
VTAG = 15  # bump to invalidate terminal-side NEFF cache (shape-keyed)
KWPAD = 128  # patch partition dim padded 122->128: DMA splits across
             # gcd(outer_dim, 16) engines, so 122 -> only 2 engines


def _ap(base, free_off, dims):
    """Custom AP: base tile slice (sets partition range), explicit free dims."""
    return bass.AP(tensor=base.tensor, offset=base.offset + free_off,
                   ap=[base.ap[0]] + [list(d) for d in dims])


def _plan(top_indices):
    """Shared (cross-core) packing plan from top_indices."""
    ti = np.asarray(top_indices, np.int64)
    B, K, R = ti.shape
    counts = np.zeros((8, 256), np.int64)
    for q in range(8):
        b, crow = q // 4, q % 4
        cells = ti[b].reshape(-1)
        sel = cells[(cells >= crow * 256) & (cells < (crow + 1) * 256)] - crow * 256
        counts[q] = np.bincount(sel, minlength=256)
    order = np.argsort(counts, axis=1, kind='stable')  # per core: rank -> cell
    srt = np.sort(counts, axis=1)
    cap = srt.max(0)                      # capacity per rank
    cap = np.maximum(cap, 1)              # every rank owns >= 1 slot
    # interleave small/large cells so per-chunk segment count (= tensor
    # matmul load) is even across chunks instead of front-loaded; the
    # per-rank majorization (srt[q][perm[i]] <= cap[perm[i]]) still holds
    perm = np.empty(256, np.int64)
    perm[0::2] = np.arange(128)
    perm[1::2] = np.arange(128, 256)
    order = order[:, perm]
    cap = cap[perm]
    bnd = np.cumsum(cap)
    S2 = int(bnd[-1])
    NCH = (S2 + 127) // 128
    SP = NCH * 128
    # segments per chunk: (rank, s0_in_chunk, n)
    segs = [[] for _ in range(NCH)]
    for r in range(256):
        s0, s1 = int(bnd[r] - cap[r]), int(bnd[r])
        for ch in range(s0 // 128, (s1 - 1) // 128 + 1):
            a = max(s0, ch * 128)
            b_ = min(s1, (ch + 1) * 128)
            segs[ch].append((r, a - ch * 128, b_ - a))
    # extend final segment to cover padding tail
    if S2 < SP:
        r, a, n = segs[-1][-1]
        segs[-1][-1] = (r, a, n + SP - S2)
    return dict(counts=counts, order=order, cap=cap, bnd=bnd, S2=S2,
                SP=SP, NCH=NCH, segs=segs)


def _build_module(plan):
    SP, NCH, segs = plan['SP'], plan['NCH'], plan['segs']
    nsegtot = sum(len(s) for s in segs)
    nc = bacc.Bacc("TRN2", target_bir_lowering=False, debug=False)
    dt = nc.dram_tensor
    uinT = dt("uinT", [512, SP], BF16, kind="ExternalInput")
    pblob = dt("pblob", [KWPAD, nsegtot * D], BF16, kind="ExternalInput")
    wu = dt("wu", [416, D], BF16, kind="ExternalInput")
    wub = dt("wub", [D, 1], F32, kind="ExternalInput")
    wdaA = dt("wdaA", [96, 240], BF16, kind="ExternalInput")
    wdaB = dt("wdaB", [97, 240], BF16, kind="ExternalInput")
    bda = dt("bda", [112, 1], F32, kind="ExternalInput")
    bdb = dt("bdb", [32, 1], F32, kind="ExternalInput")
    blog = dt("blog", [72, 1], F32, kind="ExternalInput")
    bd6 = dt("bd6", [72, 72], BF16, kind="ExternalInput")
    sgA = dt("sgA", [112, 1], F32, kind="ExternalInput")
    sgB = dt("sgB", [32, 1], F32, kind="ExternalInput")
    iotah = dt("iotah", [128, 2 * HATB + VTAG], BF16, kind="ExternalInput")
    onesw = dt("onesw", [96, 96], BF16, kind="ExternalInput")
    identf = dt("identf", [128, 128], F32, kind="ExternalInput")
    identb = dt("identb", [128, 128], BF16, kind="ExternalInput")
    woA = dt("woA", [96, D], BF16, kind="ExternalInput")
    woB = dt("woB", [96, D], BF16, kind="ExternalInput")
    wob = dt("wob", [1, D], BF16, kind="ExternalInput")
    outT = dt("outT", [D, SP], F32, kind="ExternalOutput")

    NCS = [(i * 512, min(512, SP - i * 512)) for i in range((SP + 511) // 512)]
    AF = mybir.ActivationFunctionType
    OP = mybir.AluOpType

    with tile.TileContext(nc) as tc:
        with (
            tc.tile_pool(name="const", bufs=1) as cpool,
            tc.tile_pool(name="big", bufs=1) as bpool,
        ):
            _sbn = [0]
            def sb(t_ap, shape, dtype):
                _sbn[0] += 1
                nm = f"cst{_sbn[0]}"
                x = cpool.tile(shape, dtype, tag=nm, name=nm)
                nc.scalar.dma_start(x[:], t_ap)
                return x

            s_wu = []
            for kc in range(4):
                k0, k1 = kc * 128, min((kc + 1) * 128, 416)
                s_wu.append(sb(wu[k0:k1, :], [k1 - k0, D], BF16))
            s_wub = [sb(wub[0:96, :], [96, 1], F32), sb(wub[96:192, :], [96, 1], F32)]
            s_wdaA = sb(wdaA[:], [96, 240], BF16)
            s_wdaB = sb(wdaB[:], [97, 240], BF16)
            s_bda = sb(bda[:], [112, 1], F32)
            s_bdb = sb(bdb[:], [32, 1], F32)
            s_blog = sb(blog[:], [72, 1], F32)
            s_bd6 = sb(bd6[:], [72, 72], BF16)
            s_sgA = sb(sgA[:], [112, 1], F32)
            s_sgB = sb(sgB[:], [32, 1], F32)
            s_iota = sb(iotah[0:128, 0:2 * HATB], [128, 2 * HATB], BF16)
            s_ones = sb(onesw[:], [96, 96], BF16)
            s_idf = sb(identf[:], [128, 128], F32)
            s_idb = sb(identb[:], [128, 128], BF16)
            s_woA = sb(woA[:], [96, D], BF16)
            s_woB = sb(woB[:], [96, D], BF16)
            s_wob = sb(wob[:], [1, D], BF16)
            s_eps = cpool.tile([96, 1], F32, name="s_eps")
            nc.vector.memset(s_eps[:], 1e-5)
            s_one1 = cpool.tile([1, 128], BF16, name="s_one1")
            nc.vector.memset(s_one1[:], 1.0)

            # persistent activations
            yP = [bpool.tile([96, SP], BF16, tag="yP0", name="yP0"),
                  bpool.tile([96, SP], BF16, tag="yP1", name="yP1")]
            muP = bpool.tile([96, SP], F32, tag="muP")
            varP = bpool.tile([96, SP], F32, tag="varP")
            u0 = bpool.tile([96, SP], BF16, tag="u0", name="u0")
            u1 = bpool.tile([97, SP], BF16, tag="u1", name="u1")
            xaP = bpool.tile([112, SP], F32, tag="xaP")
            xbP = bpool.tile([32, SP], F32, tag="xbP")
            xwP = bpool.tile([72, SP], F32, tag="xwP")

            # ======== pass A: u matmul, gelu, stats  (gelu act table) ========
            with (
                tc.tile_pool(name="ucp", bufs=3) as ucpool,
                tc.tile_pool(name="wkA", bufs=2) as wpool,
                tc.tile_pool(name="psA", bufs=2, space="PSUM") as psA,
                tc.tile_pool(name="psB", bufs=2, space="PSUM") as psB,
            ):
                for n0, nn in NCS:
                    uc = ucpool.tile([128, 4, 512], BF16, tag="uc")
                    nc.sync.dma_start(
                        uc[:, :, :nn],
                        bass.AP(tensor=uinT[:].tensor, offset=n0,
                                ap=[[SP, 128], [128 * SP, 4], [1, nn]]))
                    pu = psA.tile([96, 2, 512], F32, tag="pu")
                    for mc in range(2):
                        for kc in range(4):
                            kk = min(128, 416 - kc * 128)
                            nc.tensor.matmul(
                                pu[:, mc, :nn],
                                s_wu[kc][:, mc * 96:(mc + 1) * 96],
                                uc[:kk, kc, :nn],
                                start=(kc == 0), stop=(kc == 3))
                        nc.scalar.activation(
                            out=yP[mc][:, n0:n0 + nn], in_=pu[:, mc, :nn],
                            func=AF.Gelu, bias=s_wub[mc], scale=1.0)
                    y2 = wpool.tile([96, 2, 512], BF16, tag="y2")
                    for mc in range(2):
                        nc.gpsimd.tensor_mul(
                            y2[:, mc, :nn], yP[mc][:, n0:n0 + nn],
                            yP[mc][:, n0:n0 + nn])
                    pst = psB.tile([96, 2, 512], F32, tag="pst")
                    nc.tensor.matmul(pst[:, 0, :nn], s_ones[:],
                                     yP[0][:, n0:n0 + nn], start=True, stop=False)
                    nc.tensor.matmul(pst[:, 0, :nn], s_ones[:],
                                     yP[1][:, n0:n0 + nn], start=False, stop=True)
                    nc.tensor.matmul(pst[:, 1, :nn], s_ones[:],
                                     y2[:, 0, :nn], start=True, stop=False)
                    nc.tensor.matmul(pst[:, 1, :nn], s_ones[:],
                                     y2[:, 1, :nn], start=False, stop=True)
                    nc.vector.tensor_scalar_mul(
                        out=muP[:, n0:n0 + nn], in0=pst[:, 0, :nn],
                        scalar1=1.0 / D)
                    musq = wpool.tile([96, 512], F32, tag="musq")
                    nc.gpsimd.tensor_mul(musq[:, :nn], muP[:, n0:n0 + nn],
                                         muP[:, n0:n0 + nn])
                    nc.vector.scalar_tensor_tensor(
                        out=varP[:, n0:n0 + nn], in0=pst[:, 1, :nn],
                        scalar=1.0 / D, in1=musq[:, :nn],
                        op0=OP.mult, op1=OP.subtract)

            # ======== pass B: rr = 1/sqrt(var+eps)  (sqrt act table) ========
            with tc.tile_pool(name="wkB", bufs=2) as wpool:
                for n0, nn in NCS:
                    sd = wpool.tile([96, 512], F32, tag="sd")
                    nc.scalar.activation(out=sd[:, :nn],
                                         in_=varP[:, n0:n0 + nn],
                                         func=AF.Sqrt, bias=s_eps, scale=1.0)
                    nc.vector.reciprocal_approx_fast(
                        out=varP[:, n0:n0 + nn], in_=sd[:, :nn])

            # ======== pass C (per 512 block) interleaved with phase F ========
            with (
                tc.tile_pool(name="wkC", bufs=2) as wpool,
                tc.tile_pool(name="psC", bufs=1, space="PSUM") as psC,
                tc.tile_pool(name="psD2", bufs=1, space="PSUM") as psD2,
                tc.tile_pool(name="psE", bufs=1, space="PSUM") as psE,
                tc.tile_pool(name="kw", bufs=4) as kpool,
                tc.tile_pool(name="pp", bufs=6) as ppool,
                tc.tile_pool(name="psT", bufs=1, space="PSUM") as psT,
                tc.tile_pool(name="psK", bufs=1, space="PSUM") as psK,
                tc.tile_pool(name="psX", bufs=1, space="PSUM") as psX,
                tc.tile_pool(name="psDo", bufs=1, space="PSUM") as psDo,
            ):
                def passC(n0, nn):
                    nc.vector.tensor_mul(u0[:, n0:n0 + nn],
                                         yP[0][:, n0:n0 + nn],
                                         varP[:, n0:n0 + nn])
                    nc.gpsimd.tensor_mul(u1[0:96, n0:n0 + nn],
                                         yP[1][:, n0:n0 + nn],
                                         varP[:, n0:n0 + nn])
                    nc.vector.tensor_mul(u1[96:97, n0:n0 + nn],
                                         muP[0:1, n0:n0 + nn],
                                         varP[0:1, n0:n0 + nn])
                    pdc = psC.tile([112, 512], F32, tag="pdc")
                    nc.tensor.matmul(pdc[:, :nn], s_wdaA[:, 0:112],
                                     u0[:, n0:n0 + nn], start=True, stop=False)
                    nc.tensor.matmul(pdc[:, :nn], s_wdaB[:, 0:112],
                                     u1[:, n0:n0 + nn], start=False, stop=True)
                    pdd = psD2.tile([128, 512], F32, tag="pdd")
                    nc.tensor.matmul(pdd[:, :nn], s_wdaA[:, 112:240],
                                     u0[:, n0:n0 + nn], start=True, stop=False)
                    nc.tensor.matmul(pdd[:, :nn], s_wdaB[:, 112:240],
                                     u1[:, n0:n0 + nn], start=False, stop=True)
                    nc.scalar.activation(out=xaP[:, n0:n0 + nn],
                                         in_=pdc[:, :nn],
                                         func=AF.Tanh, bias=s_bda, scale=1.0)
                    nc.scalar.activation(out=xbP[:, n0:n0 + nn],
                                         in_=pdd[96:128, :nn],
                                         func=AF.Tanh, bias=s_bdb, scale=1.0)
                    nc.scalar.activation(out=xaP[:, n0:n0 + nn],
                                         in_=xaP[:, n0:n0 + nn],
                                         func=AF.Copy, scale=s_sgA)
                    nc.scalar.activation(out=xbP[:, n0:n0 + nn],
                                         in_=xbP[:, n0:n0 + nn],
                                         func=AF.Copy, scale=s_sgB)
                    exw = wpool.tile([72, 512], BF16, tag="exw")
                    nc.scalar.activation(out=exw[:, :nn], in_=pdd[0:72, :nn],
                                         func=AF.Exp, bias=s_blog, scale=1.0)
                    pz = psE.tile([72, 512], F32, tag="pz")
                    nc.tensor.matmul(pz[:, :nn], s_bd6[:], exw[:, :nn],
                                     start=True, stop=True)
                    rz = wpool.tile([72, 512], F32, tag="rz")
                    nc.vector.reciprocal_approx_fast(out=rz[:, :nn],
                                                     in_=pz[:, :nn])
                    nc.vector.tensor_mul(xwP[:, n0:n0 + nn], exw[:, :nn],
                                         rz[:, :nn])

                # ======== phase F: hats, kappa, sampling, w_o ========
                pcolv = [0]
                def S1(q):
                    c0 = q * 128
                    sg = segs[q]
                    nseg = len(sg)
                    st = {}
                    pT = psT.tile([128, 216], F32, tag="pT", name="pT")
                    nc.tensor.transpose(pT[:, 0:112], xaP[:, c0:c0 + 128],
                                        s_idf[:112, :112])
                    nc.tensor.transpose(pT[:, 112:144], xbP[:, c0:c0 + 128],
                                        s_idf[:32, :32])
                    nc.tensor.transpose(pT[:, 144:216], xwP[:, c0:c0 + 128],
                                        s_idf[:72, :72])
                    rm = kpool.tile([128, 216], F32, tag="rm", name="rm")
                    nc.scalar.copy(out=rm[:], in_=pT[:])
                    patch = ppool.tile([KWPAD, nseg * D], BF16, tag="patch",
                                       name="patch")
                    pcol = pcolv[0]
                    nc.sync.dma_start(patch[:],
                                      pblob[:, pcol * D:(pcol + nseg) * D])
                    pcolv[0] += nseg
                    hxy = kpool.tile([128, 2 * HATB], F32, tag="hxy",
                                     name="hxy")
                    for coord in range(2):
                        eng = nc.vector if coord == 0 else nc.gpsimd
                        for l in range(NL):
                            w = WXY[l]
                            out_ap = _ap(hxy[:], coord * HATB + HOFF[l],
                                         [[72, 6], [w, 4], [1, w]])
                            in0 = _ap(rm[:], 8 * l + coord,
                                      [[24, 6], [2, 4], [0, w]])
                            in1 = _ap(s_iota[:], coord * HATB + HOFF[l],
                                      [[72, 6], [w, 4], [1, w]])
                            eng.tensor_sub(out_ap, in0, in1)
                    st['rm'], st['hxy'], st['patch'] = rm, hxy, patch
                    return st

                def S2a(q, st):
                    hs = st['hxy'][:]
                    nc.scalar.activation(out=hs, in_=hs, func=AF.Abs)
                    nc.scalar.activation(out=hs, in_=hs, func=AF.Relu,
                                         bias=1.0, scale=-1.0)

                def S2(q, st):
                    rm, hxy = st['rm'], st['hxy']
                    tail = q >= NCH - 2   # vector drains last chunks
                    for l in range(NL):
                        w = WXY[l]
                        hy_ap = _ap(hxy[:], HATB + HOFF[l],
                                    [[72, 6], [w, 4], [1, w]])
                        wt_ap = _ap(rm[:], 144 + 4 * l,
                                    [[12, 6], [1, 4], [0, w]])
                        eng = nc.gpsimd if (l == 0 and not tail) else nc.vector
                        eng.tensor_mul(hy_ap, hy_ap, wt_ap)
                    kap = kpool.tile([128, 6 * KWIN], BF16, tag="kap",
                                     name="kap")
                    tmp = kpool.tile([128, 6 * 4 * WXY[0] ** 2], F32,
                                     tag="tmp", name="tmp")
                    for l in range(NL):
                        w = WXY[l]
                        for m in range(4):
                            hy = _ap(hxy[:], HATB + HOFF[l] + m * w,
                                     [[72, 6], [1, w], [0, w]])
                            hx = _ap(hxy[:], HOFF[l] + m * w,
                                     [[72, 6], [0, w], [1, w]])
                            t1 = _ap(tmp[:], m * w * w,
                                     [[4 * w * w, 6], [w, w], [1, w]])
                            eng = (nc.gpsimd if m == 3 else nc.vector
                                   ) if tail else (
                                   nc.gpsimd if m % 2 else nc.vector)
                            eng.tensor_mul(t1, hy, hx)
                        t2a = _ap(tmp[:], 0,
                                  [[4 * w * w, 6], [w * w, 2], [w, w], [1, w]])
                        t2b = _ap(tmp[:], 2 * w * w,
                                  [[4 * w * w, 6], [w * w, 2], [w, w], [1, w]])
                        eng = nc.vector if (l == 0 or tail) else nc.gpsimd
                        eng.tensor_add(t2a, t2a, t2b)
                        ksl = _ap(kap[:], LOFF[l], [[KWIN, 6], [w, w], [1, w]])
                        t1a = _ap(tmp[:], 0, [[4 * w * w, 6], [w, w], [1, w]])
                        t1b = _ap(tmp[:], w * w,
                                  [[4 * w * w, 6], [w, w], [1, w]])
                        eng = nc.vector if tail else (
                            nc.gpsimd if l == 0 else nc.vector)
                        eng.tensor_add(ksl, t1a, t1b)
                    st['kap'] = kap

                def S3(q, st):
                    c0 = q * 128
                    sg = segs[q]
                    kap, patch = st['kap'], st['patch']
                    pK = psK.tile([122, 6, 128], BF16, tag="pK", name="pK")
                    for hh in range(H):
                        nc.tensor.transpose(pK[:, hh, :],
                                            kap[:, hh * KWIN:(hh + 1) * KWIN],
                                            s_idb[:])
                    kT = kpool.tile([122, 6, 128], BF16, tag="kT", name="kT")
                    nc.scalar.copy(out=kT[:, 0:3, :], in_=pK[:, 0:3, :])
                    nc.vector.tensor_copy(kT[:, 3:6, :], pK[:, 3:6, :])
                    pXt = psX.tile([96, 8, 128], F32, tag="pXt", name="pXt")
                    pXa = pXt[:, 0:3, :]
                    pXb = pXt[:, 4:7, :]
                    for j, (r, s0, n) in enumerate(sg):
                        nc.tensor.matmul(
                            pXa[:, :, s0:s0 + n],
                            patch[0:KWIN, j * D:j * D + 96],
                            kT[:, 0:3, s0:s0 + n],
                            start=True, stop=True)
                        nc.tensor.matmul(
                            pXb[:, :, s0:s0 + n],
                            patch[0:KWIN, j * D + 96:j * D + 192],
                            kT[:, 3:6, s0:s0 + n],
                            start=True, stop=True)
                    XU = kpool.tile([96, 2, 128], BF16, tag="XU", name="XU")
                    for hh in range(3):
                        base = pXt[32 * hh:32 * hh + 32, 0, :]
                        nc.scalar.copy(
                            out=XU[32 * hh:32 * hh + 32, :, :],
                            in_=_ap(base, hh * 128, [[512, 2], [1, 128]]))
                    pDt = psDo.tile([96, 2, 128], F32, tag="pDt", name="pDt")
                    od = kpool.tile([96, 2, 128], F32, tag="od", name="od")
                    for mc in range(2):
                        nc.tensor.matmul(pDt[:, mc, :],
                                         s_woA[:, mc * 96:(mc + 1) * 96],
                                         XU[:, 0, :], start=True, stop=False)
                        nc.tensor.matmul(pDt[:, mc, :],
                                         s_woB[:, mc * 96:(mc + 1) * 96],
                                         XU[:, 1, :], start=False, stop=False)
                        nc.tensor.matmul(pDt[:, mc, :],
                                         s_wob[:, mc * 96:(mc + 1) * 96],
                                         s_one1[:], start=False, stop=True)
                    nc.scalar.copy(out=od[:], in_=pDt[:])
                    nc.sync.dma_start(
                        bass.AP(tensor=outT[:].tensor, offset=c0,
                                ap=[[SP, 96], [96 * SP, 2], [1, 128]]),
                        od[:])

                # interleave pass C blocks with a 2-stage software skew of
                # phase F (engine queues are in-order; interleaving chunks
                # fills cross-engine handoff bubbles, and starting F right
                # after C(0) overlaps the lead-in)
                nblk = len(NCS)
                blk_end = [(n0 + nn) // 128 for n0, nn in NCS]
                emitted_c = [0]
                def needC(t):
                    while emitted_c[0] < nblk and (
                            0 if emitted_c[0] == 0
                            else blk_end[emitted_c[0] - 1]) < t + 1:
                        j = emitted_c[0]
                        passC(NCS[j][0], NCS[j][1])
                        emitted_c[0] += 1
                sts = [None] * NCH
                for t in range(NCH + 3):
                    if 0 <= t - 2 < NCH:
                        S2a(t - 2, sts[t - 2])
                    if t < NCH:
                        needC(min(t + 1, NCH - 1))
                        sts[t] = S1(t)
                    if 0 <= t - 2 < NCH:
                        S2(t - 2, sts[t - 2])
                    if t - 3 >= 0:
                        S3(t - 3, sts[t - 3])
    nc.compile()
    return nc


def _host_prep(inputs, plan):
    h = inputs["h"].astype(np.float32)
    ti = np.asarray(inputs["top_indices"], np.int64)
    qc = inputs["query_coords"].astype(np.float32)
    g = inputs["g"].astype(np.float32)
    maps = [np.asarray(inputs["L2_proj"], np.float32),
            np.asarray(inputs["L3_proj"], np.float32),
            np.asarray(inputs["L4_proj"], np.float32)]
    B, K, R = ti.shape
    cap, bnd, SP, NCH, segs = (plan['cap'], plan['bnd'], plan['SP'],
                               plan['NCH'], plan['segs'])
    order = plan['order']

    consts = {}
    consts["wu"] = np.ascontiguousarray(inputs["w_u_w"].T).astype(ml_dtypes.bfloat16)
    consts["wub"] = inputs["w_u_b"].reshape(D, 1).astype(np.float32)
    # LN fold: z = Wg.(y*rr) - rowsum(Wg).(mu*rr) + (W.b + c)
    gam = inputs["ln_u_g"].astype(np.float32)
    bet = inputs["ln_u_b"].astype(np.float32)
    Wall = np.concatenate([inputs["w_delta_w"], inputs["w_a_w"]], 0)  # [216,192]
    ball = np.concatenate([inputs["w_delta_b"], inputs["w_a_b"]], 0)  # [216]
    Wg = Wall * gam[None, :]
    Wg240 = np.zeros((240, D), np.float32)
    Wg240[0:112] = Wg[0:112]
    Wg240[112:184] = Wg[144:216]
    Wg240[208:240] = Wg[112:144]
    lhs = np.concatenate([Wg240.T, -Wg240.sum(1)[None, :]], 0)  # [193, 240]
    consts["wdaA"] = lhs[0:96].astype(ml_dtypes.bfloat16)
    consts["wdaB"] = lhs[96:193].astype(ml_dtypes.bfloat16)
    biasf = Wall @ bet + ball                              # [216]
    consts["bda"] = biasf[0:112].reshape(112, 1).astype(np.float32)
    consts["bdb"] = biasf[112:144].reshape(32, 1).astype(np.float32)
    consts["blog"] = biasf[144:216].reshape(72, 1).astype(np.float32)
    consts["bd6"] = np.kron(np.eye(H, dtype=np.float32),
                            np.ones((12, 12), np.float32)).astype(ml_dtypes.bfloat16)
    # per-offset-row sigma (rows (h,l,m,c): l = (o//8)%3)
    sv = np.array([SIGMAS[(o // 8) % 3] for o in range(144)], np.float32)
    consts["sgA"] = sv[0:112].reshape(112, 1)
    consts["sgB"] = sv[112:144].reshape(32, 1)
    # iota: (i - clo); device x = sig*tanh, so hat = relu(1-|x - iota|)
    io = np.zeros((128, 2 * HATB + VTAG), np.float32)
    for coord in range(2):
        for l in range(NL):
            w = WXY[l]
            for hh in range(H):
                for m in range(M):
                    st = coord * HATB + HOFF[l] + 72 * hh + w * m
                    io[:, st:st + w] = np.arange(w, dtype=np.float32) - CLO[l]
    consts["iotah"] = io.astype(ml_dtypes.bfloat16)
    consts["onesw"] = np.ones((96, 96), ml_dtypes.bfloat16)
    consts["identf"] = np.eye(128, dtype=np.float32)
    consts["identb"] = np.eye(128, dtype=ml_dtypes.bfloat16)
    woT = np.ascontiguousarray(inputs["w_o_w"].T).astype(np.float32)
    consts["woA"] = woT[0:96].astype(ml_dtypes.bfloat16)
    consts["woB"] = woT[96:192].astype(ml_dtypes.bfloat16)
    consts["wob"] = (inputs["w_o_b"] + inputs["e_deform"].reshape(-1)
                     ).reshape(1, D).astype(ml_dtypes.bfloat16)

    pmaps = []
    for b in range(B):
        pm = []
        for l in range(NL):
            Wl = maps[l].shape[3]
            mp = np.transpose(maps[l][b], (1, 2, 0))
            Hp = 32 * SCALE[l] + WXY[l]
            out = np.zeros((Hp, Hp, D), np.float32)
            out[PADL[l]:PADL[l] + Wl, PADL[l]:PADL[l] + Wl] = mp
            pm.append(out.astype(ml_dtypes.bfloat16))
        pmaps.append(pm)

    freqs = 2.0 ** np.arange(NF, dtype=np.float32)
    cell_of = ti.reshape(B, K * R)

    in_maps, slot_maps = [], []
    for q in range(8):
        b, crow = q // 4, q % 4
        d = dict(consts)
        # rank -> cell (local id in 0..255), patches in per-chunk segment order
        r2c = order[q]
        cells_seq = []
        for ch in range(NCH):
            for (r, s0, n) in segs[ch]:
                cells_seq.append(r2c[r])
        pats_all = np.zeros((KWPAD, len(cells_seq) * D), ml_dtypes.bfloat16)
        for j, cid in enumerate(cells_seq):
            ayc, axc = cid // 32, cid % 32
            col = []
            for l in range(NL):
                w = WXY[l]
                pm = pmaps[b][l]
                r0 = SCALE[l] * 8 * crow
                ys = (r0 + SCALE[l] * ayc) + np.arange(w)
                xs = (SCALE[l] * axc) + np.arange(w)
                pt = pm[ys[:, None], xs[None, :], :]     # [w, w, D]
                col.append(pt.reshape(w * w, D))
            pats_all[:KWIN, j * D:(j + 1) * D] = np.concatenate(col, 0)
        d["pblob"] = pats_all

        # slot -> token
        slot_tok = -np.ones(SP, np.int64)
        cnt = plan['counts'][q]
        for r in range(256):
            cid = r2c[r]
            gcid = crow * 256 + cid
            toks = np.nonzero(cell_of[b] == gcid)[0]
            s0 = int(bnd[r] - cap[r])
            assert len(toks) <= cap[r]
            slot_tok[s0:s0 + len(toks)] = toks
        valid = slot_tok >= 0
        st = np.where(valid, slot_tok, 0)
        k_of = st // R
        cid_of = cell_of[b][st]
        h_s = h[b][k_of] * valid[:, None]
        g_s = g[b][cid_of] * valid[:, None]
        qc_s = qc[b][k_of]
        ax = (cid_of % 32).astype(np.float32)
        ay = (cid_of // 32).astype(np.float32)
        anchor = np.stack([ax * 32 + 16, ay * 32 + 16], -1)
        dp = (anchor - qc_s) / 1024.0
        xf = dp[:, 0:1] * freqs * 2 * np.pi
        yf = dp[:, 1:2] * freqs * 2 * np.pi
        phi = np.concatenate([np.sin(xf), np.cos(xf), np.sin(yf), np.cos(yf)],
                             -1).astype(np.float32) * valid[:, None]
        u_in = np.concatenate([h_s, g_s, phi], -1)
        uT = np.zeros((512, SP), ml_dtypes.bfloat16)
        uT[0:416] = np.ascontiguousarray(u_in.T).astype(ml_dtypes.bfloat16)
        d["uinT"] = uT
        in_maps.append(d)
        slot_maps.append((slot_tok, valid))
    return in_maps, slot_maps


def kernel(**inputs):
    plan = _plan(inputs["top_indices"])
    key = plan['SP'], tuple(plan['cap'].tolist())
    if _CACHE.get("key") != key:
        _CACHE["nc"] = _build_module(plan)
        _CACHE["key"] = key
    nc = _CACHE["nc"]
    in_maps, slot_maps = _host_prep(inputs, plan)
    res = run_bass_kernel_spmd(nc, in_maps, core_ids=list(range(8)),
                               **_CACHE.get("run_kwargs", {}))
    _CACHE["last"] = res
    B, K, R = inputs["top_indices"].shape
    out = np.zeros((B, K * R, D), np.float32)
    for q in range(8):
        b = q // 4
        oT = np.asarray(res.results[q]["outT"], np.float32)
        slot_tok, valid = slot_maps[q]
        out[b, slot_tok[valid]] = oT.T[valid]
    return out.reshape(B, K, R, D)
